# revision 32
# baseline (speedup 1.0000x reference)
"""Trainium2 Bass kernel for BaselineKNNModel (cosine-sim KNN classifier).

Contract: kernel(**inputs) takes FULL inputs (x [2048,512] f32,
embeddings [100000,512] f32, labels [100000] int) and returns the FULL
output (pred [2048] labels.dtype), distributing work across 8 NeuronCores.

Strategy (database-parallel, per sharding hint):
 - Host: normalize embeddings (cosine denominator), pad N 100000->102400,
   transpose to [512, N]; shard along N across 8 cores (12800 each).
   x normalization is skipped: per-query positive scaling cannot change
   that query's top-k ranking.
 - Device (SPMD, per core): sim tile [128 q, 512 c] = xT.T @ enT chunk via
   PE accumulation over K=512; per tile, VectorE max/max_index extract the
   top-8 values + indices of each 512-candidate chunk (global top-10 of a
   row is contained in the union of its per-chunk top-8s unless >=9 of the
   top-10 fall in one 512-chunk: P ~ 1e-11).
 - Host: merge 8 cores x 25 chunks x top-8 = 1600 candidates/query, exact
   top-10 by (value desc, index asc) = jax.lax.top_k tie order, then the
   reference's mode computation.
"""
import sys

for _p in ("/opt/trn_rl_repo", "/root/.axon_site/_ro/trn_rl_repo"):
    if _p not in sys.path:
        sys.path.insert(0, _p)

import numpy as np

import concourse.bacc as bacc
import concourse.mybir as mybir
import concourse.tile as tile
from concourse import bass_utils

F32 = mybir.dt.float32
F32R = mybir.dt.float32r
F16 = mybir.dt.float16
U32 = mybir.dt.uint32
Copy = mybir.ActivationFunctionType.Copy

B = 2048            # queries
D = 512             # embedding dim
N_EMB = 100000      # database size
K_NEIGH = 10
NUM_CLASSES = 1000
EPS = 1e-8

CORES = 8
N_PAD = 102400      # padded database size (8 * 12800)
N_CORE = N_PAD // CORES     # 12800 candidates per core
CHUNK = 512                 # candidates per sim tile (one PSUM bank)
NCHUNK = N_CORE // CHUNK    # 25
QT = B // 128               # 16 query tiles
KT = D // 128               # 4 k-tiles
NOUT = NCHUNK * 8           # 200 output slots per query per core

# f16w variant: window-max + device window top-16 + host exact rescore
WWIN = 32                   # candidates per window
WPC = N_CORE // WWIN        # 400 windows per core
BIGCHUNK = 1024             # candidates per PSUM tile (2 banks)
NSEL = 16                   # windows kept per (query, core, half)
HALF_A = (7 * BIGCHUNK) // WWIN  # windows in selection half A (224)
MARGIN = 4e-3               # fp16-sim error margin on unit-normalized sims
                            # (measured max |fp16 sim err| ~6e-5, ~60x safety)

# f8w variant: same as f16w but fp8e4m3 DoubleRow matmuls (2 fp8 weights per
# PE cell, K=256 per matmul). Inputs are scaled by F8_SCALE before rounding
# to fp8, so device sims (and window maxes) are scaled by F8_SCALE^2.
F8_SCALE = 16.0
MARGIN_F8 = 2.5e-2          # fp8 margin on unit-normalized sims
                            # (measured max err 7.1e-3 on a sample, rms 1.6e-3)

MM_DTYPE = "f16w"           # "f32" | "f32r" | "f16x3" | "f16w"

_CACHE = {}


def _build(variant):
    """Build + compile the per-core Bass program. Same program on all cores;
    only the `ent*` input shards differ."""
    nc = bacc.Bacc("TRN2", target_bir_lowering=False, debug=False)

    if variant == "noop":  # minimal program for RPC-overhead baselining
        d_nin = nc.dram_tensor("nin", [128, 128], F32, kind="ExternalInput")
        d_nout = nc.dram_tensor("nout", [128, 128], F32, kind="ExternalOutput")
        with tile.TileContext(nc) as tc:
            with tc.tile_pool(name="np0", bufs=1) as pool:
                t = pool.tile([128, 128], F32, tag="t")
                nc.sync.dma_start(t[:, :], d_nin[:, :])
                nc.sync.dma_start(d_nout[:, :], t[:, :])
        nc.compile()
        return nc

    if variant == "f16w":
        return _build_f16w(nc)
    if variant == "f8w":
        return _build_f8w(nc)

    f16 = variant == "f16x3"
    if f16:
        d_xhi = nc.dram_tensor("xhi", [D, B], F16, kind="ExternalInput")
        d_xlo = nc.dram_tensor("xlo", [D, B], F16, kind="ExternalInput")
        d_ehi = nc.dram_tensor("ehi", [D, N_CORE], F16, kind="ExternalInput")
        d_elo = nc.dram_tensor("elo", [D, N_CORE], F16, kind="ExternalInput")
    else:
        in_dt = F32R if variant == "f32r" else F32
        d_xt = nc.dram_tensor("xt", [D, B], in_dt, kind="ExternalInput")
        d_ent = nc.dram_tensor("ent", [D, N_CORE], in_dt, kind="ExternalInput")

    d_vals = nc.dram_tensor("vals", [B, NOUT], F32, kind="ExternalOutput")
    d_idx = nc.dram_tensor("idx", [B, NOUT], U32, kind="ExternalOutput")

    with tile.TileContext(nc) as tc:
        with (
            tc.tile_pool(name="xpool", bufs=1) as xpool,
            tc.tile_pool(name="epool", bufs=3) as epool,
            tc.tile_pool(name="ps", bufs=6, space="PSUM") as ps_pool,
            tc.tile_pool(name="sim", bufs=6) as sim_pool,
            tc.tile_pool(name="acc", bufs=1) as acc_pool,
        ):
            # resident x (stationary operand), k-tiles side by side
            if f16:
                xhi_sb = xpool.tile([128, KT * B], F16, tag="xhi")
                xlo_sb = xpool.tile([128, KT * B], F16, tag="xlo")
                for k in range(KT):
                    nc.sync.dma_start(xhi_sb[:, k * B:(k + 1) * B],
                                      d_xhi[k * 128:(k + 1) * 128, :])
                    nc.sync.dma_start(xlo_sb[:, k * B:(k + 1) * B],
                                      d_xlo[k * 128:(k + 1) * 128, :])
            else:
                xt_sb = xpool.tile([128, KT * B], in_dt, tag="xt")
                for k in range(KT):
                    nc.sync.dma_start(xt_sb[:, k * B:(k + 1) * B],
                                      d_xt[k * 128:(k + 1) * 128, :])

            # result accumulators, [128, QT*NOUT], column q*NOUT + c*8 + j
            vals_sb = acc_pool.tile([128, QT * NOUT], F32, tag="vacc")
            idx_sb = acc_pool.tile([128, QT * NOUT], U32, tag="iacc")

            for c in range(NCHUNK):
                c0 = c * CHUNK
                if f16:
                    ehi_sb = epool.tile([128, KT * CHUNK], F16, tag="ehi")
                    elo_sb = epool.tile([128, KT * CHUNK], F16, tag="elo")
                    for k in range(KT):
                        nc.sync.dma_start(ehi_sb[:, k * CHUNK:(k + 1) * CHUNK],
                                          d_ehi[k * 128:(k + 1) * 128, c0:c0 + CHUNK])
                        nc.sync.dma_start(elo_sb[:, k * CHUNK:(k + 1) * CHUNK],
                                          d_elo[k * 128:(k + 1) * 128, c0:c0 + CHUNK])
                else:
                    en_sb = epool.tile([128, KT * CHUNK], in_dt, tag="en")
                    for k in range(KT):
                        nc.sync.dma_start(en_sb[:, k * CHUNK:(k + 1) * CHUNK],
                                          d_ent[k * 128:(k + 1) * 128, c0:c0 + CHUNK])

                for q in range(QT):
                    ps = ps_pool.tile([128, CHUNK], F32, tag="ps")
                    if variant == "f16x3":
                        nmm = 3 * KT
                        i = 0
                        for k in range(KT):
                            xh = xhi_sb[:, k * B + q * 128: k * B + (q + 1) * 128]
                            xl = xlo_sb[:, k * B + q * 128: k * B + (q + 1) * 128]
                            eh = ehi_sb[:, k * CHUNK:(k + 1) * CHUNK]
                            el = elo_sb[:, k * CHUNK:(k + 1) * CHUNK]
                            for (a, bb) in ((xh, eh), (xh, el), (xl, eh)):
                                nc.tensor.matmul(ps[:, :], a, bb,
                                                 start=(i == 0), stop=(i == nmm - 1))
                                i += 1
                    else:
                        for k in range(KT):
                            lhsT = xt_sb[:, k * B + q * 128: k * B + (q + 1) * 128]
                            rhs = en_sb[:, k * CHUNK:(k + 1) * CHUNK]
                            nc.tensor.matmul(ps[:, :], lhsT, rhs,
                                             start=(k == 0), stop=(k == KT - 1))

                    sim = sim_pool.tile([128, CHUNK], F32, tag="sim")
                    nc.scalar.activation(sim[:, :], ps[:, :], Copy)

                    o = q * NOUT + c * 8
                    nc.vector.max(vals_sb[:, o:o + 8], sim[:, :])
                    nc.vector.max_index(idx_sb[:, o:o + 8], vals_sb[:, o:o + 8],
                                        sim[:, :])

            for q in range(QT):
                nc.sync.dma_start(d_vals[q * 128:(q + 1) * 128, :],
                                  vals_sb[:, q * NOUT:(q + 1) * NOUT])
                nc.sync.dma_start(d_idx[q * 128:(q + 1) * 128, :],
                                  idx_sb[:, q * NOUT:(q + 1) * NOUT])

    nc.compile()
    return nc


def _build_f16w(nc):
    """fp16 single-pass matmul; per-tile 16-wide window max (DVE reduce,
    PSUM-direct); per-core-half top-16 windows per query via
    max/match_replace (first half's selection overlaps the main loop);
    host rescores the selected windows exactly."""
    Max = mybir.AluOpType.max
    X = mybir.AxisListType.X

    d_xh = nc.dram_tensor("xh", [D, B], F16, kind="ExternalInput")
    d_eh = nc.dram_tensor("eh", [D, N_CORE], F16, kind="ExternalInput")
    d_wvals = nc.dram_tensor("wvals", [B, 2 * NSEL], F32, kind="ExternalOutput")
    d_widx = nc.dram_tensor("widx", [B, 2 * NSEL], U32, kind="ExternalOutput")

    # chunk layout: 12 x 1024 + 1 x 512 = 12800
    chunks = [(i * BIGCHUNK, BIGCHUNK) for i in range(N_CORE // BIGCHUNK)]
    rem = N_CORE - (N_CORE // BIGCHUNK) * BIGCHUNK
    if rem:
        chunks.append((N_CORE - rem, rem))
    # selection halves aligned to chunk boundaries:
    # half A = chunks 0-6 (448 windows), half B = chunks 7-12 (352 windows)
    HALF_B = WPC - HALF_A

    def select(wq, vout, iout, o, width, mr_pool):
        nc.vector.max(vout[:, o:o + 8], wq)
        nc.vector.max_index(iout[:, o:o + 8], vout[:, o:o + 8], wq)
        mr = mr_pool.tile([128, width], F32, tag="mr")
        nc.vector.match_replace(mr[:, :width], vout[:, o:o + 8], wq, -1e30)
        nc.vector.max(vout[:, o + 8:o + 16], mr[:, :width])
        nc.vector.max_index(iout[:, o + 8:o + 16],
                            vout[:, o + 8:o + 16], mr[:, :width])

    with tile.TileContext(nc) as tc:
        with (
            tc.tile_pool(name="xpool", bufs=1) as xpool,
            tc.tile_pool(name="epool", bufs=3) as epool,
            tc.tile_pool(name="ps", bufs=3, space="PSUM") as ps_pool,
            tc.tile_pool(name="wacc", bufs=1) as wacc_pool,
            tc.tile_pool(name="mrp", bufs=4) as mr_pool,
            tc.tile_pool(name="outp", bufs=1) as out_pool,
        ):
            xh_sb = xpool.tile([128, KT * B], F16, tag="xh")
            for k in range(KT):
                nc.sync.dma_start(xh_sb[:, k * B:(k + 1) * B],
                                  d_xh[k * 128:(k + 1) * 128, :])

            wmax_sb = wacc_pool.tile([128, QT * WPC], F32, tag="wacc")
            vout_sb = out_pool.tile([128, QT * 2 * NSEL], F32, tag="vout")
            iout_sb = out_pool.tile([128, QT * 2 * NSEL], U32, tag="iout")

            for ci, (c0, cw) in enumerate(chunks):
                eh_sb = epool.tile([128, KT * BIGCHUNK], F16, tag="eh")
                for k in range(KT):
                    nc.sync.dma_start(eh_sb[:, k * cw:(k + 1) * cw],
                                      d_eh[k * 128:(k + 1) * 128, c0:c0 + cw])
                for q in range(QT):
                    ps = ps_pool.tile([128, BIGCHUNK], F32, tag="ps")
                    for s in range(cw // 512):
                        for k in range(KT):
                            nc.tensor.matmul(
                                ps[:, s * 512:(s + 1) * 512],
                                xh_sb[:, k * B + q * 128: k * B + (q + 1) * 128],
                                eh_sb[:, k * cw + s * 512: k * cw + s * 512 + 512],
                                start=(k == 0), stop=(k == KT - 1))
                    nwin = cw // WWIN
                    wslot = q * WPC + c0 // WWIN
                    nc.vector.tensor_reduce(
                        wmax_sb[:, wslot:wslot + nwin],
                        ps[:, :cw].rearrange("p (w i) -> p w i", i=WWIN),
                        axis=X, op=Max)
                # half A (windows [0, HALF_A)) is complete after chunk 6;
                # spread its per-q selection over chunks 6..12 (2-3 q each)
                if ci >= 6:
                    n_grp = len(chunks) - 6
                    qs = [q for q in range(QT) if q % n_grp == ci - 6]
                    for q in qs:
                        select(wmax_sb[:, q * WPC:q * WPC + HALF_A],
                               vout_sb, iout_sb, q * 2 * NSEL, HALF_A, mr_pool)

            for q in range(QT):  # half B (windows [HALF_A, WPC))
                select(wmax_sb[:, q * WPC + HALF_A:(q + 1) * WPC],
                       vout_sb, iout_sb, q * 2 * NSEL + NSEL, HALF_B, mr_pool)

            for q in range(QT):
                nc.sync.dma_start(d_wvals[q * 128:(q + 1) * 128, :],
                                  vout_sb[:, q * 2 * NSEL:(q + 1) * 2 * NSEL])
                nc.sync.dma_start(d_widx[q * 128:(q + 1) * 128, :],
                                  iout_sb[:, q * 2 * NSEL:(q + 1) * 2 * NSEL])

    nc.compile()
    return nc


def _build_f8w(nc):
    """Same structure as f16w, but fp8e4m3 DoubleRow matmuls: operands carry
    [partition, j(2), cols] APs; each matmul contracts 256 dims (2 k-groups
    of 128), so K=512 takes 2 matmuls per 512-wide output slice."""
    Max = mybir.AluOpType.max
    X = mybir.AxisListType.X
    F8 = mybir.dt.float8e4
    DR = mybir.MatmulPerfMode.DoubleRow

    d_x8 = nc.dram_tensor("x8", [D, B], F8, kind="ExternalInput")
    d_e8 = nc.dram_tensor("e8", [D, N_CORE], F8, kind="ExternalInput")
    d_wvals = nc.dram_tensor("wvals", [B, 2 * NSEL], F32, kind="ExternalOutput")
    d_widx = nc.dram_tensor("widx", [B, 2 * NSEL], U32, kind="ExternalOutput")

    chunks = [(i * BIGCHUNK, BIGCHUNK) for i in range(N_CORE // BIGCHUNK)]
    rem = N_CORE - (N_CORE // BIGCHUNK) * BIGCHUNK
    if rem:
        chunks.append((N_CORE - rem, rem))
    HALF_B = WPC - HALF_A

    def select(wq, vout, iout, o, width, mr_pool):
        nc.vector.max(vout[:, o:o + 8], wq)
        nc.vector.max_index(iout[:, o:o + 8], vout[:, o:o + 8], wq)
        mr = mr_pool.tile([128, width], F32, tag="mr")
        nc.vector.match_replace(mr[:, :width], vout[:, o:o + 8], wq, -1e30)
        nc.vector.max(vout[:, o + 8:o + 16], mr[:, :width])
        nc.vector.max_index(iout[:, o + 8:o + 16],
                            vout[:, o + 8:o + 16], mr[:, :width])

    with tile.TileContext(nc) as tc:
        with (
            tc.tile_pool(name="xpool", bufs=1) as xpool,
            tc.tile_pool(name="epool", bufs=3) as epool,
            tc.tile_pool(name="ps", bufs=3, space="PSUM") as ps_pool,
            tc.tile_pool(name="wacc", bufs=1) as wacc_pool,
            tc.tile_pool(name="mrp", bufs=4) as mr_pool,
            tc.tile_pool(name="outp", bufs=1) as out_pool,
        ):
            # [g][j][cols] layout: row-range g*256 + j*128 of the [D, *] input
            x_sb = xpool.tile([128, 4 * B], F8, tag="x8")
            for g in range(2):
                for j in range(2):
                    r0 = g * 256 + j * 128
                    nc.sync.dma_start(x_sb[:, (g * 2 + j) * B:(g * 2 + j + 1) * B],
                                      d_x8[r0:r0 + 128, :])

            wmax_sb = wacc_pool.tile([128, QT * WPC], F32, tag="wacc")
            vout_sb = out_pool.tile([128, QT * 2 * NSEL], F32, tag="vout")
            iout_sb = out_pool.tile([128, QT * 2 * NSEL], U32, tag="iout")

            for ci, (c0, cw) in enumerate(chunks):
                eh_sb = epool.tile([128, 4 * BIGCHUNK], F8, tag="e8")
                for g in range(2):
                    for j in range(2):
                        r0 = g * 256 + j * 128
                        nc.sync.dma_start(
                            eh_sb[:, (g * 2 + j) * cw:(g * 2 + j + 1) * cw],
                            d_e8[r0:r0 + 128, c0:c0 + cw])
                for q in range(QT):
                    ps = ps_pool.tile([128, BIGCHUNK], F32, tag="ps")
                    for s in range(cw // 512):
                        for g in range(2):
                            lhsT = x_sb[:, g * 2 * B:(g + 1) * 2 * B].rearrange(
                                "p (j b) -> p j b", j=2)[:, :, q * 128:(q + 1) * 128]
                            rhs = eh_sb[:, g * 2 * cw:(g + 1) * 2 * cw].rearrange(
                                "p (j n) -> p j n", j=2)[:, :, s * 512:(s + 1) * 512]
                            nc.tensor.matmul(ps[:, s * 512:(s + 1) * 512],
                                             lhsT, rhs, perf_mode=DR,
                                             start=(g == 0), stop=(g == 1))
                    nwin = cw // WWIN
                    wslot = q * WPC + c0 // WWIN
                    nc.vector.tensor_reduce(
                        wmax_sb[:, wslot:wslot + nwin],
                        ps[:, :cw].rearrange("p (w i) -> p w i", i=WWIN),
                        axis=X, op=Max)
                if ci >= 6:
                    n_grp = len(chunks) - 6
                    qs = [q for q in range(QT) if q % n_grp == ci - 6]
                    for q in qs:
                        select(wmax_sb[:, q * WPC:q * WPC + HALF_A],
                               vout_sb, iout_sb, q * 2 * NSEL, HALF_A, mr_pool)

            for q in range(QT):
                select(wmax_sb[:, q * WPC + HALF_A:(q + 1) * WPC],
                       vout_sb, iout_sb, q * 2 * NSEL + NSEL, HALF_B, mr_pool)

            for q in range(QT):
                nc.sync.dma_start(d_wvals[q * 128:(q + 1) * 128, :],
                                  vout_sb[:, q * 2 * NSEL:(q + 1) * 2 * NSEL])
                nc.sync.dma_start(d_widx[q * 128:(q + 1) * 128, :],
                                  iout_sb[:, q * 2 * NSEL:(q + 1) * 2 * NSEL])

    nc.compile()
    return nc


_F8_LUT = None


def _to_f8(a):
    """Fast float->fp8e4m3: fp16 hardware cast, then a 64K-entry LUT over the
    fp16 bit patterns (ml_dtypes' elementwise astype is ~50x slower). The
    double rounding vs a direct fp32->fp8 cast is harmless here: any
    consistent rounding is covered by the selection margin."""
    global _F8_LUT
    import ml_dtypes
    if _F8_LUT is None:
        all16 = np.arange(65536, dtype=np.uint16).view(np.float16)
        _F8_LUT = (all16.astype(np.float32)
                   .astype(ml_dtypes.float8_e4m3).view(np.uint8))
    h = a.astype(np.float16).view(np.uint16)
    return _F8_LUT[h].view(ml_dtypes.float8_e4m3)


def _prep_f8w(xn, e, inv):
    """in_maps for the f8w variant: fp8e4m3 transposed normalized shards,
    scaled by F8_SCALE to stay clear of the fp8 subnormal range."""
    import ml_dtypes
    f8 = ml_dtypes.float8_e4m3
    x8 = _to_f8(np.ascontiguousarray(xn.T) * np.float32(F8_SCALE))
    in_maps = []
    for i in range(CORES):
        lo_r, hi_r = i * N_CORE, (i + 1) * N_CORE
        n_real = max(0, min(hi_r, N_EMB) - lo_r)
        e8 = np.zeros((D, N_CORE), dtype=f8)
        if n_real > 0:
            sl = e[lo_r:lo_r + n_real] * (inv[lo_r:lo_r + n_real]
                                          * np.float32(F8_SCALE))[:, None]
            e8[:, :n_real] = _to_f8(sl.T)
        in_maps.append({"x8": x8, "e8": e8})
    return in_maps


def _get_nc(variant=None):
    variant = variant or MM_DTYPE
    if variant not in _CACHE:
        _CACHE[variant] = _build(variant)
    return _CACHE[variant]


def _normalize(x, embeddings):
    x = np.asarray(x, dtype=np.float32)
    e = np.asarray(embeddings, dtype=np.float32)
    xn = x / np.maximum(np.linalg.norm(x, axis=1, keepdims=True), EPS)
    inv = (1.0 / np.maximum(np.linalg.norm(e, axis=1), EPS)).astype(np.float32)
    return xn, e, inv


def _prep_f16w(xn, e, inv):
    """in_maps for the f16w variant: fp16 transposed normalized shards."""
    xh = np.ascontiguousarray(xn.T).astype(np.float16)
    in_maps = []
    for i in range(CORES):
        lo_r, hi_r = i * N_CORE, (i + 1) * N_CORE
        n_real = max(0, min(hi_r, N_EMB) - lo_r)
        eh = np.zeros((D, N_CORE), dtype=np.float16)
        if n_real > 0:
            sl = e[lo_r:lo_r + n_real] * inv[lo_r:lo_r + n_real][:, None]
            eh[:, :n_real] = sl.T.astype(np.float16)
        in_maps.append({"xh": xh, "eh": eh})
    return in_maps


def _prep_inputs(x, embeddings, variant):
    """Host prep: normalize embeddings, pad, transpose, shard; returns in_maps.

    Works per-core-shard to keep intermediates cache-sized."""
    if variant == "f16w":
        xn, e, inv = _normalize(x, embeddings)
        return _prep_f16w(xn, e, inv)
    if variant == "f8w":
        xn, e, inv = _normalize(x, embeddings)
        return _prep_f8w(xn, e, inv)
    x = np.asarray(x, dtype=np.float32)
    e = np.asarray(embeddings, dtype=np.float32)
    inv = (1.0 / np.maximum(np.linalg.norm(e, axis=1), EPS)).astype(np.float32)
    xt = np.ascontiguousarray(x.T)               # [D, B]

    in_maps = []
    for i in range(CORES):
        lo_r, hi_r = i * N_CORE, (i + 1) * N_CORE
        n_real = max(0, min(hi_r, N_EMB) - lo_r)
        ent = np.zeros((D, N_CORE), dtype=np.float32)
        if n_real > 0:
            sl = e[lo_r:lo_r + n_real]
            ent[:, :n_real] = sl.T * inv[lo_r:lo_r + n_real][None, :]
        if variant == "f16x3":
            ehi = ent.astype(np.float16)
            elo = (ent - ehi).astype(np.float16)
            in_maps.append({"ehi": ehi, "elo": elo})
        else:
            in_maps.append({"ent": ent})

    if variant == "f16x3":
        xhi = xt.astype(np.float16)
        xlo = (xt - xhi).astype(np.float16)
        for m in in_maps:
            m["xhi"] = xhi
            m["xlo"] = xlo
    else:
        for m in in_maps:
            m["xt"] = xt
    return in_maps


def _merge(results, labels):
    """Host merge: exact global top-10 from per-core per-chunk top-8 pools,
    then the reference's mode computation."""
    vals = np.concatenate([r["vals"] for r in results], axis=1)   # [B, 8*NOUT]
    idx8 = np.concatenate([r["idx"] for r in results], axis=1).astype(np.int64)

    col_base = (np.arange(NOUT, dtype=np.int64) // 8) * CHUNK      # chunk offset
    core_base = np.repeat(np.arange(CORES, dtype=np.int64) * N_CORE, NOUT)
    g = idx8 + np.tile(col_base, CORES)[None, :] + core_base[None, :]

    # padding rows (g >= N_EMB) are zero embeddings: exclude
    u = vals.view(np.uint32)
    key = np.where(u & 0x80000000, ~u, u | 0x80000000).astype(np.uint64)
    combo = ((np.uint64(0xFFFFFFFF) - key) << np.uint64(17)) | g.astype(np.uint64)
    combo[g >= N_EMB] = np.uint64(0xFFFFFFFFFFFFFFFF)
    order = np.argsort(combo, axis=1, kind="stable")[:, :K_NEIGH]
    neighbors = np.take_along_axis(g, order, axis=1)               # [B, 10]

    labels = np.asarray(labels)
    nl = labels[neighbors].astype(np.int64)                        # [B, 10]
    eq = nl[:, :, None] == nl[:, None, :]
    counts = eq.sum(-1)
    mkey = counts * (NUM_CLASSES + 1) + (NUM_CLASSES - nl)
    mi = np.argmax(mkey, axis=1)
    pred = np.take_along_axis(nl, mi[:, None], axis=1)[:, 0]
    return pred.astype(labels.dtype)


class _Runner:
    """Caches the shard_map-jitted executable across calls (mirrors
    bass2jax.run_bass_via_pjrt's multi-core path, which re-traces per call)."""

    def __init__(self, variant):
        import jax
        import concourse.mybir as mb
        from concourse import bass2jax
        from jax.experimental.shard_map import shard_map
        from jax.sharding import Mesh, PartitionSpec

        bass2jax.install_neuronx_cc_hook()
        self.jax = jax
        nc = _get_nc(variant)
        partition_name = (nc.partition_id_tensor.name
                          if nc.partition_id_tensor else None)
        in_names, out_names, out_avals, zeros = [], [], [], []
        for alloc in nc.m.functions[0].allocations:
            if not isinstance(alloc, mb.MemoryLocationSet):
                continue
            name = alloc.memorylocations[0].name
            if alloc.kind == "ExternalInput":
                if name != partition_name:
                    in_names.append(name)
            elif alloc.kind == "ExternalOutput":
                shape = tuple(alloc.tensor_shape)
                dtype = mb.dt.np(alloc.dtype)
                out_avals.append(jax.core.ShapedArray(shape, dtype))
                out_names.append(name)
                zeros.append(np.zeros((CORES * shape[0],) + shape[1:], dtype))
        self.in_names = list(in_names)
        self.out_names = out_names
        self.out_avals = out_avals
        self.zeros = zeros
        n_params = len(in_names)
        all_names = in_names + out_names
        if partition_name is not None:
            all_names = all_names + [partition_name]
        donate = tuple(range(n_params, n_params + len(out_names)))

        def _body(*args):
            operands = list(args)
            if partition_name is not None:
                operands.append(bass2jax.partition_id_tensor())
            outs = bass2jax._bass_exec_p.bind(
                *operands,
                out_avals=tuple(out_avals),
                in_names=tuple(all_names),
                out_names=tuple(out_names),
                lowering_input_output_aliases=(),
                sim_require_finite=True,
                sim_require_nnan=True,
                nc=nc,
            )
            return tuple(outs)

        devices = jax.devices()[:CORES]
        self.mesh = Mesh(np.asarray(devices), ("core",))
        self.pspec = PartitionSpec("core")
        in_specs = (self.pspec,) * (n_params + len(out_names))
        out_specs = (self.pspec,) * len(out_names)
        self.sharded = jax.jit(
            shard_map(_body, mesh=self.mesh, in_specs=in_specs,
                      out_specs=out_specs, check_rep=False),
            donate_argnums=donate, keep_unused=True,
        )

    def concat_inputs(self, in_maps):
        return [
            np.concatenate([np.asarray(m[name]) for m in in_maps], axis=0)
            for name in self.in_names
        ]

    def device_put(self, concat_in):
        from jax.sharding import NamedSharding
        sh = NamedSharding(self.mesh, self.pspec)
        return [self.jax.device_put(a, sh) for a in concat_in]

    def execute(self, concat_in):
        zeros = [np.zeros_like(z) for z in self.zeros]
        out_arrs = self.sharded(*concat_in, *zeros)
        return out_arrs

    def run(self, in_maps):
        out_arrs = self.execute(self.concat_inputs(in_maps))
        return [
            {
                name: np.asarray(out_arrs[i]).reshape(
                    CORES, *self.out_avals[i].shape)[c]
                for i, name in enumerate(self.out_names)
            }
            for c in range(CORES)
        ]


_RUNNERS = {}


def _get_runner(variant=None):
    variant = variant or MM_DTYPE
    if variant not in _RUNNERS:
        _RUNNERS[variant] = _Runner(variant)
    return _RUNNERS[variant]


def _mode_pred(neighbors, labels):
    """Reference's torch.mode semantics on gathered neighbor labels."""
    labels = np.asarray(labels)
    nl = labels[neighbors].astype(np.int64)                        # [B, 10]
    eq = nl[:, :, None] == nl[:, None, :]
    counts = eq.sum(-1)
    mkey = counts * (NUM_CLASSES + 1) + (NUM_CLASSES - nl)
    mi = np.argmax(mkey, axis=1)
    pred = np.take_along_axis(nl, mi[:, None], axis=1)[:, 0]
    return pred.astype(labels.dtype)


def _merge_f16w(results, labels, xn, e, inv, margin=MARGIN):
    """Select windows >= (10th-best window max) - margin, rescore those
    candidates exactly in fp64, exact global top-10, then mode."""
    wv = np.stack([r["wvals"] for r in results], axis=1)      # [B, 8, 32]
    wi = np.stack([r["widx"] for r in results], axis=1).astype(np.int64)
    wi[:, :, NSEL:] += HALF_A   # half-B indices are relative to its slice
    gw = wi + (np.arange(CORES, dtype=np.int64) * WPC)[None, :, None]
    wv = wv.reshape(B, CORES * 2 * NSEL)
    gw = gw.reshape(B, CORES * 2 * NSEL)

    w10 = np.partition(wv, wv.shape[1] - K_NEIGH, axis=1)[:, wv.shape[1] - K_NEIGH]
    keep = wv >= (w10[:, None] - margin)
    smax = int(keep.sum(axis=1).max())

    # top-smax windows per row by value; mask out ones below the cutoff
    order = np.argsort(-wv, axis=1, kind="stable")[:, :smax]
    sel_g = np.take_along_axis(gw, order, axis=1)              # [B, smax]
    sel_keep = np.take_along_axis(keep, order, axis=1)

    # rescore grouped by window: each window's embeddings are one contiguous
    # 32-row slice, shared by every query that selected it (~6400 windows
    # total vs ~170k (row, window) pairs -> tiny gathers, BLAS-sized GEMMs)
    e = np.asarray(e, dtype=np.float32)
    xn32 = np.ascontiguousarray(xn, dtype=np.float32)
    rows_idx, slots = np.nonzero(sel_keep)
    wins = sel_g[rows_idx, slots]
    order = np.argsort(wins, kind="stable")
    rows_idx, slots, wins = rows_idx[order], slots[order], wins[order]
    uniq, starts = np.unique(wins, return_index=True)
    bounds = np.append(starts, len(wins))

    sims = np.full((B, smax, WWIN), -np.inf, dtype=np.float32)
    for ui in range(len(uniq)):
        w = int(uniq[ui])
        c0, c1 = w * WWIN, min(w * WWIN + WWIN, N_EMB)
        if c1 <= c0:
            continue
        s0, s1 = bounds[ui], bounds[ui + 1]
        en_w = e[c0:c1] * inv[c0:c1][:, None]                  # [<=32, D]
        sblk = xn32[rows_idx[s0:s1]] @ en_w.T                  # [nrows, <=32]
        sims[rows_idx[s0:s1], slots[s0:s1], :c1 - c0] = sblk

    cand = (sel_g[:, :, None] * WWIN +
            np.arange(WWIN, dtype=np.int64)[None, None, :]).reshape(B, -1)
    sims = sims.reshape(B, -1)

    # exact top-10 by (-sim, cand) via an order-preserving uint64 key
    u = sims.view(np.uint32)
    mono = np.where(u & 0x80000000, ~u, u | 0x80000000).astype(np.uint64)
    combo = ((np.uint64(0xFFFFFFFF) - mono) << np.uint64(17)) | \
        cand.astype(np.uint64)
    combo[sims == -np.inf] = np.uint64(0xFFFFFFFFFFFFFFFF)
    ordr = np.argsort(combo, axis=1, kind="stable")[:, :K_NEIGH]
    neighbors = np.take_along_axis(cand, ordr, axis=1)
    return _mode_pred(neighbors, labels)


def run_on_hw(x, embeddings, variant=None):
    runner = _get_runner(variant)
    in_maps = _prep_inputs(x, embeddings, variant or MM_DTYPE)
    return runner.run(in_maps)


def kernel(x, embeddings, labels):
    variant = MM_DTYPE
    if variant == "f16w":
        xn, e, inv = _normalize(x, embeddings)
        runner = _get_runner(variant)
        results = runner.run(_prep_f16w(xn, e, inv))
        return _merge_f16w(results, labels, xn, e, inv)
    if variant == "f8w":
        xn, e, inv = _normalize(x, embeddings)
        runner = _get_runner(variant)
        results = runner.run(_prep_f8w(xn, e, inv))
        return _merge_f16w(results, labels, xn, e, inv,
                           margin=MARGIN_F8 * F8_SCALE * F8_SCALE)
    results = run_on_hw(x, embeddings)
    return _merge(results, labels)


# revision 39
# speedup vs baseline: 1.2353x; 1.2353x over previous
"""Trainium2 Bass kernel for BaselineKNNModel (cosine-sim KNN classifier).

Contract: kernel(**inputs) takes FULL inputs (x [2048,512] f32,
embeddings [100000,512] f32, labels [100000] int) and returns the FULL
output (pred [2048] labels.dtype), distributing work across 8 NeuronCores.

Strategy (database-parallel, per sharding hint):
 - Host: normalize embeddings (cosine denominator), pad N 100000->102400,
   transpose to [512, N]; shard along N across 8 cores (12800 each).
   x normalization is skipped: per-query positive scaling cannot change
   that query's top-k ranking.
 - Device (SPMD, per core): sim tile [128 q, 512 c] = xT.T @ enT chunk via
   PE accumulation over K=512; per tile, VectorE max/max_index extract the
   top-8 values + indices of each 512-candidate chunk (global top-10 of a
   row is contained in the union of its per-chunk top-8s unless >=9 of the
   top-10 fall in one 512-chunk: P ~ 1e-11).
 - Host: merge 8 cores x 25 chunks x top-8 = 1600 candidates/query, exact
   top-10 by (value desc, index asc) = jax.lax.top_k tie order, then the
   reference's mode computation.
"""
import sys

for _p in ("/opt/trn_rl_repo", "/root/.axon_site/_ro/trn_rl_repo"):
    if _p not in sys.path:
        sys.path.insert(0, _p)

import numpy as np

import concourse.bacc as bacc
import concourse.mybir as mybir
import concourse.tile as tile
from concourse import bass_utils

F32 = mybir.dt.float32
F32R = mybir.dt.float32r
F16 = mybir.dt.float16
U32 = mybir.dt.uint32
Copy = mybir.ActivationFunctionType.Copy

B = 2048            # queries
D = 512             # embedding dim
N_EMB = 100000      # database size
K_NEIGH = 10
NUM_CLASSES = 1000
EPS = 1e-8

CORES = 8
N_PAD = 102400      # padded database size (8 * 12800)
N_CORE = N_PAD // CORES     # 12800 candidates per core
CHUNK = 512                 # candidates per sim tile (one PSUM bank)
NCHUNK = N_CORE // CHUNK    # 25
QT = B // 128               # 16 query tiles
KT = D // 128               # 4 k-tiles
NOUT = NCHUNK * 8           # 200 output slots per query per core

# f16w variant: window-max + device window top-16 + host exact rescore
WWIN = 32                   # candidates per window
WPC = N_CORE // WWIN        # 400 windows per core
BIGCHUNK = 1024             # candidates per PSUM tile (2 banks)
NSEL = 16                   # windows kept per (query, core, half)
HALF_A = (7 * BIGCHUNK) // WWIN  # windows in selection half A (224)
MARGIN = 4e-3               # fp16-sim error margin on unit-normalized sims
                            # (measured max |fp16 sim err| ~6e-5, ~60x safety)

# f8w variant: same as f16w but fp8e4m3 DoubleRow matmuls (2 fp8 weights per
# PE cell, K=256 per matmul). Inputs are scaled by F8_SCALE before rounding
# to fp8, so device sims (and window maxes) are scaled by F8_SCALE^2.
F8_SCALE = 16.0
MARGIN_F8 = 2.5e-2          # fp8 margin on unit-normalized sims
                            # (measured max err 7.1e-3 on a sample, rms 1.6e-3)

MM_DTYPE = "f8w"            # "f32" | "f32r" | "f16x3" | "f16w" | "f8w"

_CACHE = {}


def _build(variant):
    """Build + compile the per-core Bass program. Same program on all cores;
    only the `ent*` input shards differ."""
    nc = bacc.Bacc("TRN2", target_bir_lowering=False, debug=False)

    if variant == "noop":  # minimal program for RPC-overhead baselining
        d_nin = nc.dram_tensor("nin", [128, 128], F32, kind="ExternalInput")
        d_nout = nc.dram_tensor("nout", [128, 128], F32, kind="ExternalOutput")
        with tile.TileContext(nc) as tc:
            with tc.tile_pool(name="np0", bufs=1) as pool:
                t = pool.tile([128, 128], F32, tag="t")
                nc.sync.dma_start(t[:, :], d_nin[:, :])
                nc.sync.dma_start(d_nout[:, :], t[:, :])
        nc.compile()
        return nc

    if variant == "f16w":
        return _build_f16w(nc)
    if variant == "f8w":
        return _build_f8w(nc)
    if variant == "f8d":
        return _build_f8d(nc)

    f16 = variant == "f16x3"
    if f16:
        d_xhi = nc.dram_tensor("xhi", [D, B], F16, kind="ExternalInput")
        d_xlo = nc.dram_tensor("xlo", [D, B], F16, kind="ExternalInput")
        d_ehi = nc.dram_tensor("ehi", [D, N_CORE], F16, kind="ExternalInput")
        d_elo = nc.dram_tensor("elo", [D, N_CORE], F16, kind="ExternalInput")
    else:
        in_dt = F32R if variant == "f32r" else F32
        d_xt = nc.dram_tensor("xt", [D, B], in_dt, kind="ExternalInput")
        d_ent = nc.dram_tensor("ent", [D, N_CORE], in_dt, kind="ExternalInput")

    d_vals = nc.dram_tensor("vals", [B, NOUT], F32, kind="ExternalOutput")
    d_idx = nc.dram_tensor("idx", [B, NOUT], U32, kind="ExternalOutput")

    with tile.TileContext(nc) as tc:
        with (
            tc.tile_pool(name="xpool", bufs=1) as xpool,
            tc.tile_pool(name="epool", bufs=3) as epool,
            tc.tile_pool(name="ps", bufs=6, space="PSUM") as ps_pool,
            tc.tile_pool(name="sim", bufs=6) as sim_pool,
            tc.tile_pool(name="acc", bufs=1) as acc_pool,
        ):
            # resident x (stationary operand), k-tiles side by side
            if f16:
                xhi_sb = xpool.tile([128, KT * B], F16, tag="xhi")
                xlo_sb = xpool.tile([128, KT * B], F16, tag="xlo")
                for k in range(KT):
                    nc.sync.dma_start(xhi_sb[:, k * B:(k + 1) * B],
                                      d_xhi[k * 128:(k + 1) * 128, :])
                    nc.sync.dma_start(xlo_sb[:, k * B:(k + 1) * B],
                                      d_xlo[k * 128:(k + 1) * 128, :])
            else:
                xt_sb = xpool.tile([128, KT * B], in_dt, tag="xt")
                for k in range(KT):
                    nc.sync.dma_start(xt_sb[:, k * B:(k + 1) * B],
                                      d_xt[k * 128:(k + 1) * 128, :])

            # result accumulators, [128, QT*NOUT], column q*NOUT + c*8 + j
            vals_sb = acc_pool.tile([128, QT * NOUT], F32, tag="vacc")
            idx_sb = acc_pool.tile([128, QT * NOUT], U32, tag="iacc")

            for c in range(NCHUNK):
                c0 = c * CHUNK
                if f16:
                    ehi_sb = epool.tile([128, KT * CHUNK], F16, tag="ehi")
                    elo_sb = epool.tile([128, KT * CHUNK], F16, tag="elo")
                    for k in range(KT):
                        nc.sync.dma_start(ehi_sb[:, k * CHUNK:(k + 1) * CHUNK],
                                          d_ehi[k * 128:(k + 1) * 128, c0:c0 + CHUNK])
                        nc.sync.dma_start(elo_sb[:, k * CHUNK:(k + 1) * CHUNK],
                                          d_elo[k * 128:(k + 1) * 128, c0:c0 + CHUNK])
                else:
                    en_sb = epool.tile([128, KT * CHUNK], in_dt, tag="en")
                    for k in range(KT):
                        nc.sync.dma_start(en_sb[:, k * CHUNK:(k + 1) * CHUNK],
                                          d_ent[k * 128:(k + 1) * 128, c0:c0 + CHUNK])

                for q in range(QT):
                    ps = ps_pool.tile([128, CHUNK], F32, tag="ps")
                    if variant == "f16x3":
                        nmm = 3 * KT
                        i = 0
                        for k in range(KT):
                            xh = xhi_sb[:, k * B + q * 128: k * B + (q + 1) * 128]
                            xl = xlo_sb[:, k * B + q * 128: k * B + (q + 1) * 128]
                            eh = ehi_sb[:, k * CHUNK:(k + 1) * CHUNK]
                            el = elo_sb[:, k * CHUNK:(k + 1) * CHUNK]
                            for (a, bb) in ((xh, eh), (xh, el), (xl, eh)):
                                nc.tensor.matmul(ps[:, :], a, bb,
                                                 start=(i == 0), stop=(i == nmm - 1))
                                i += 1
                    else:
                        for k in range(KT):
                            lhsT = xt_sb[:, k * B + q * 128: k * B + (q + 1) * 128]
                            rhs = en_sb[:, k * CHUNK:(k + 1) * CHUNK]
                            nc.tensor.matmul(ps[:, :], lhsT, rhs,
                                             start=(k == 0), stop=(k == KT - 1))

                    sim = sim_pool.tile([128, CHUNK], F32, tag="sim")
                    nc.scalar.activation(sim[:, :], ps[:, :], Copy)

                    o = q * NOUT + c * 8
                    nc.vector.max(vals_sb[:, o:o + 8], sim[:, :])
                    nc.vector.max_index(idx_sb[:, o:o + 8], vals_sb[:, o:o + 8],
                                        sim[:, :])

            for q in range(QT):
                nc.sync.dma_start(d_vals[q * 128:(q + 1) * 128, :],
                                  vals_sb[:, q * NOUT:(q + 1) * NOUT])
                nc.sync.dma_start(d_idx[q * 128:(q + 1) * 128, :],
                                  idx_sb[:, q * NOUT:(q + 1) * NOUT])

    nc.compile()
    return nc


def _build_f16w(nc):
    """fp16 single-pass matmul; per-tile 16-wide window max (DVE reduce,
    PSUM-direct); per-core-half top-16 windows per query via
    max/match_replace (first half's selection overlaps the main loop);
    host rescores the selected windows exactly."""
    Max = mybir.AluOpType.max
    X = mybir.AxisListType.X

    d_xh = nc.dram_tensor("xh", [D, B], F16, kind="ExternalInput")
    d_eh = nc.dram_tensor("eh", [D, N_CORE], F16, kind="ExternalInput")
    d_wvals = nc.dram_tensor("wvals", [B, 2 * NSEL], F32, kind="ExternalOutput")
    d_widx = nc.dram_tensor("widx", [B, 2 * NSEL], U32, kind="ExternalOutput")

    # chunk layout: 12 x 1024 + 1 x 512 = 12800
    chunks = [(i * BIGCHUNK, BIGCHUNK) for i in range(N_CORE // BIGCHUNK)]
    rem = N_CORE - (N_CORE // BIGCHUNK) * BIGCHUNK
    if rem:
        chunks.append((N_CORE - rem, rem))
    # selection halves aligned to chunk boundaries:
    # half A = chunks 0-6 (448 windows), half B = chunks 7-12 (352 windows)
    HALF_B = WPC - HALF_A

    def select(wq, vout, iout, o, width, mr_pool):
        nc.vector.max(vout[:, o:o + 8], wq)
        nc.vector.max_index(iout[:, o:o + 8], vout[:, o:o + 8], wq)
        mr = mr_pool.tile([128, width], F32, tag="mr")
        nc.vector.match_replace(mr[:, :width], vout[:, o:o + 8], wq, -1e30)
        nc.vector.max(vout[:, o + 8:o + 16], mr[:, :width])
        nc.vector.max_index(iout[:, o + 8:o + 16],
                            vout[:, o + 8:o + 16], mr[:, :width])

    with tile.TileContext(nc) as tc:
        with (
            tc.tile_pool(name="xpool", bufs=1) as xpool,
            tc.tile_pool(name="epool", bufs=3) as epool,
            tc.tile_pool(name="ps", bufs=3, space="PSUM") as ps_pool,
            tc.tile_pool(name="wacc", bufs=1) as wacc_pool,
            tc.tile_pool(name="mrp", bufs=4) as mr_pool,
            tc.tile_pool(name="outp", bufs=1) as out_pool,
        ):
            xh_sb = xpool.tile([128, KT * B], F16, tag="xh")
            for k in range(KT):
                nc.sync.dma_start(xh_sb[:, k * B:(k + 1) * B],
                                  d_xh[k * 128:(k + 1) * 128, :])

            wmax_sb = wacc_pool.tile([128, QT * WPC], F32, tag="wacc")
            vout_sb = out_pool.tile([128, QT * 2 * NSEL], F32, tag="vout")
            iout_sb = out_pool.tile([128, QT * 2 * NSEL], U32, tag="iout")

            for ci, (c0, cw) in enumerate(chunks):
                eh_sb = epool.tile([128, KT * BIGCHUNK], F16, tag="eh")
                for k in range(KT):
                    nc.sync.dma_start(eh_sb[:, k * cw:(k + 1) * cw],
                                      d_eh[k * 128:(k + 1) * 128, c0:c0 + cw])
                for q in range(QT):
                    ps = ps_pool.tile([128, BIGCHUNK], F32, tag="ps")
                    for s in range(cw // 512):
                        for k in range(KT):
                            nc.tensor.matmul(
                                ps[:, s * 512:(s + 1) * 512],
                                xh_sb[:, k * B + q * 128: k * B + (q + 1) * 128],
                                eh_sb[:, k * cw + s * 512: k * cw + s * 512 + 512],
                                start=(k == 0), stop=(k == KT - 1))
                    nwin = cw // WWIN
                    wslot = q * WPC + c0 // WWIN
                    nc.vector.tensor_reduce(
                        wmax_sb[:, wslot:wslot + nwin],
                        ps[:, :cw].rearrange("p (w i) -> p w i", i=WWIN),
                        axis=X, op=Max)
                # half A (windows [0, HALF_A)) is complete after chunk 6;
                # spread its per-q selection over chunks 6..12 (2-3 q each)
                if ci >= 6:
                    n_grp = len(chunks) - 6
                    qs = [q for q in range(QT) if q % n_grp == ci - 6]
                    for q in qs:
                        select(wmax_sb[:, q * WPC:q * WPC + HALF_A],
                               vout_sb, iout_sb, q * 2 * NSEL, HALF_A, mr_pool)

            for q in range(QT):  # half B (windows [HALF_A, WPC))
                select(wmax_sb[:, q * WPC + HALF_A:(q + 1) * WPC],
                       vout_sb, iout_sb, q * 2 * NSEL + NSEL, HALF_B, mr_pool)

            for q in range(QT):
                nc.sync.dma_start(d_wvals[q * 128:(q + 1) * 128, :],
                                  vout_sb[:, q * 2 * NSEL:(q + 1) * 2 * NSEL])
                nc.sync.dma_start(d_widx[q * 128:(q + 1) * 128, :],
                                  iout_sb[:, q * 2 * NSEL:(q + 1) * 2 * NSEL])

    nc.compile()
    return nc


def _build_f8w(nc):
    """Same structure as f16w, but fp8e4m3 DoubleRow matmuls: operands carry
    [partition, j(2), cols] APs; each matmul contracts 256 dims (2 k-groups
    of 128), so K=512 takes 2 matmuls per 512-wide output slice."""
    Max = mybir.AluOpType.max
    X = mybir.AxisListType.X
    F8 = mybir.dt.float8e4
    DR = mybir.MatmulPerfMode.DoubleRow

    d_x8 = nc.dram_tensor("x8", [D, B], F8, kind="ExternalInput")
    d_e8 = nc.dram_tensor("e8", [D, N_CORE], F8, kind="ExternalInput")
    d_wvals = nc.dram_tensor("wvals", [B, 2 * NSEL], F32, kind="ExternalOutput")
    d_widx = nc.dram_tensor("widx", [B, 2 * NSEL], U32, kind="ExternalOutput")

    chunks = [(i * BIGCHUNK, BIGCHUNK) for i in range(N_CORE // BIGCHUNK)]
    rem = N_CORE - (N_CORE // BIGCHUNK) * BIGCHUNK
    if rem:
        chunks.append((N_CORE - rem, rem))
    HALF_B = WPC - HALF_A

    def select(wq, vout, iout, o, width, mr_pool):
        nc.vector.max(vout[:, o:o + 8], wq)
        nc.vector.max_index(iout[:, o:o + 8], vout[:, o:o + 8], wq)
        mr = mr_pool.tile([128, width], F32, tag="mr")
        nc.vector.match_replace(mr[:, :width], vout[:, o:o + 8], wq, -1e30)
        nc.vector.max(vout[:, o + 8:o + 16], mr[:, :width])
        nc.vector.max_index(iout[:, o + 8:o + 16],
                            vout[:, o + 8:o + 16], mr[:, :width])

    with tile.TileContext(nc) as tc:
        with (
            tc.tile_pool(name="xpool", bufs=1) as xpool,
            tc.tile_pool(name="epool", bufs=3) as epool,
            tc.tile_pool(name="ps", bufs=3, space="PSUM") as ps_pool,
            tc.tile_pool(name="wacc", bufs=1) as wacc_pool,
            tc.tile_pool(name="mrp", bufs=4) as mr_pool,
            tc.tile_pool(name="outp", bufs=1) as out_pool,
        ):
            # [g][j][cols] layout: row-range g*256 + j*128 of the [D, *] input
            x_sb = xpool.tile([128, 4 * B], F8, tag="x8")
            for g in range(2):
                for j in range(2):
                    r0 = g * 256 + j * 128
                    nc.sync.dma_start(x_sb[:, (g * 2 + j) * B:(g * 2 + j + 1) * B],
                                      d_x8[r0:r0 + 128, :])

            wmax_sb = wacc_pool.tile([128, QT * WPC], F32, tag="wacc")
            vout_sb = out_pool.tile([128, QT * 2 * NSEL], F32, tag="vout")
            iout_sb = out_pool.tile([128, QT * 2 * NSEL], U32, tag="iout")

            for ci, (c0, cw) in enumerate(chunks):
                eh_sb = epool.tile([128, 4 * BIGCHUNK], F8, tag="e8")
                for g in range(2):
                    for j in range(2):
                        r0 = g * 256 + j * 128
                        nc.sync.dma_start(
                            eh_sb[:, (g * 2 + j) * cw:(g * 2 + j + 1) * cw],
                            d_e8[r0:r0 + 128, c0:c0 + cw])
                for q in range(QT):
                    ps = ps_pool.tile([128, BIGCHUNK], F32, tag="ps")
                    for s in range(cw // 512):
                        for g in range(2):
                            lhsT = x_sb[:, g * 2 * B:(g + 1) * 2 * B].rearrange(
                                "p (j b) -> p j b", j=2)[:, :, q * 128:(q + 1) * 128]
                            rhs = eh_sb[:, g * 2 * cw:(g + 1) * 2 * cw].rearrange(
                                "p (j n) -> p j n", j=2)[:, :, s * 512:(s + 1) * 512]
                            nc.tensor.matmul(ps[:, s * 512:(s + 1) * 512],
                                             lhsT, rhs, perf_mode=DR,
                                             start=(g == 0), stop=(g == 1))
                    nwin = cw // WWIN
                    wslot = q * WPC + c0 // WWIN
                    nc.vector.tensor_reduce(
                        wmax_sb[:, wslot:wslot + nwin],
                        ps[:, :cw].rearrange("p (w i) -> p w i", i=WWIN),
                        axis=X, op=Max)
                if ci >= 6:
                    n_grp = len(chunks) - 6
                    qs = [q for q in range(QT) if q % n_grp == ci - 6]
                    for q in qs:
                        select(wmax_sb[:, q * WPC:q * WPC + HALF_A],
                               vout_sb, iout_sb, q * 2 * NSEL, HALF_A, mr_pool)

            for q in range(QT):
                select(wmax_sb[:, q * WPC + HALF_A:(q + 1) * WPC],
                       vout_sb, iout_sb, q * 2 * NSEL + NSEL, HALF_B, mr_pool)

            for q in range(QT):
                nc.sync.dma_start(d_wvals[q * 128:(q + 1) * 128, :],
                                  vout_sb[:, q * 2 * NSEL:(q + 1) * 2 * NSEL])
                nc.sync.dma_start(d_widx[q * 128:(q + 1) * 128, :],
                                  iout_sb[:, q * 2 * NSEL:(q + 1) * 2 * NSEL])

    nc.compile()
    return nc


_F8_LUT = None


def _to_f8(a):
    """Fast float->fp8e4m3: fp16 hardware cast, then a 64K-entry LUT over the
    fp16 bit patterns (ml_dtypes' elementwise astype is ~50x slower). The
    double rounding vs a direct fp32->fp8 cast is harmless here: any
    consistent rounding is covered by the selection margin."""
    global _F8_LUT
    import ml_dtypes
    if _F8_LUT is None:
        with np.errstate(all="ignore"):
            all16 = np.arange(65536, dtype=np.uint16).view(np.float16)
            _F8_LUT = (all16.astype(np.float32)
                       .astype(ml_dtypes.float8_e4m3).view(np.uint8))
    h = a.astype(np.float16).view(np.uint16)
    return _F8_LUT[h].view(ml_dtypes.float8_e4m3)


def _build_f8d(nc):
    """f8w minus on-device window selection: the full per-window max array
    ships to the host (3.3MB/core), which does the margin selection itself.
    ScalarE stages PSUM->SBUF so the DVE reduce pays the SBUF (not PSUM)
    access bubble; DVE runs nothing but the 208 window-max reduces."""
    Max = mybir.AluOpType.max
    X = mybir.AxisListType.X
    F8 = mybir.dt.float8e4
    DR = mybir.MatmulPerfMode.DoubleRow
    Copy = mybir.ActivationFunctionType.Copy

    d_x8 = nc.dram_tensor("x8", [D, B], F8, kind="ExternalInput")
    d_e8 = nc.dram_tensor("e8", [D, N_CORE], F8, kind="ExternalInput")
    d_wmax = nc.dram_tensor("wmax", [B, WPC], F32, kind="ExternalOutput")

    chunks = [(i * BIGCHUNK, BIGCHUNK) for i in range(N_CORE // BIGCHUNK)]
    rem = N_CORE - (N_CORE // BIGCHUNK) * BIGCHUNK
    if rem:
        chunks.append((N_CORE - rem, rem))

    with tile.TileContext(nc) as tc:
        with (
            tc.tile_pool(name="xpool", bufs=1) as xpool,
            tc.tile_pool(name="epool", bufs=3) as epool,
            tc.tile_pool(name="ps", bufs=3, space="PSUM") as ps_pool,
            tc.tile_pool(name="stg", bufs=3) as stg_pool,
            tc.tile_pool(name="wacc", bufs=1) as wacc_pool,
        ):
            x_sb = xpool.tile([128, 4 * B], F8, tag="x8")
            for g in range(2):
                for j in range(2):
                    r0 = g * 256 + j * 128
                    nc.sync.dma_start(x_sb[:, (g * 2 + j) * B:(g * 2 + j + 1) * B],
                                      d_x8[r0:r0 + 128, :])

            wmax_sb = wacc_pool.tile([128, QT * WPC], F32, tag="wacc")

            for (c0, cw) in chunks:
                eh_sb = epool.tile([128, 4 * BIGCHUNK], F8, tag="e8")
                for g in range(2):
                    for j in range(2):
                        r0 = g * 256 + j * 128
                        nc.sync.dma_start(
                            eh_sb[:, (g * 2 + j) * cw:(g * 2 + j + 1) * cw],
                            d_e8[r0:r0 + 128, c0:c0 + cw])
                for q in range(QT):
                    ps = ps_pool.tile([128, BIGCHUNK], F32, tag="ps")
                    for s in range(cw // 512):
                        for g in range(2):
                            lhsT = x_sb[:, g * 2 * B:(g + 1) * 2 * B].rearrange(
                                "p (j b) -> p j b", j=2)[:, :, q * 128:(q + 1) * 128]
                            rhs = eh_sb[:, g * 2 * cw:(g + 1) * 2 * cw].rearrange(
                                "p (j n) -> p j n", j=2)[:, :, s * 512:(s + 1) * 512]
                            nc.tensor.matmul(ps[:, s * 512:(s + 1) * 512],
                                             lhsT, rhs, perf_mode=DR,
                                             start=(g == 0), stop=(g == 1))
                    stg = stg_pool.tile([128, BIGCHUNK], F32, tag="stg")
                    nc.scalar.activation(stg[:, :cw], ps[:, :cw], Copy)
                    nwin = cw // WWIN
                    wslot = q * WPC + c0 // WWIN
                    nc.vector.tensor_reduce(
                        wmax_sb[:, wslot:wslot + nwin],
                        stg[:, :cw].rearrange("p (w i) -> p w i", i=WWIN),
                        axis=X, op=Max)

            for q in range(QT):
                nc.sync.dma_start(d_wmax[q * 128:(q + 1) * 128, :],
                                  wmax_sb[:, q * WPC:(q + 1) * WPC])

    nc.compile()
    return nc


def _prep_f8w(xn, e, inv):
    """in_maps for the f8w variant: fp8e4m3 transposed normalized shards,
    scaled by F8_SCALE to stay clear of the fp8 subnormal range."""
    import ml_dtypes
    f8 = ml_dtypes.float8_e4m3
    x8 = _to_f8(np.ascontiguousarray(xn.T) * np.float32(F8_SCALE))
    in_maps = []
    for i in range(CORES):
        lo_r, hi_r = i * N_CORE, (i + 1) * N_CORE
        n_real = max(0, min(hi_r, N_EMB) - lo_r)
        e8 = np.zeros((D, N_CORE), dtype=f8)
        if n_real > 0:
            sl = e[lo_r:lo_r + n_real] * (inv[lo_r:lo_r + n_real]
                                          * np.float32(F8_SCALE))[:, None]
            e8[:, :n_real] = _to_f8(sl.T)
        in_maps.append({"x8": x8, "e8": e8})
    return in_maps


def _get_nc(variant=None):
    variant = variant or MM_DTYPE
    if variant not in _CACHE:
        _CACHE[variant] = _build(variant)
    return _CACHE[variant]


def _normalize(x, embeddings):
    x = np.asarray(x, dtype=np.float32)
    e = np.asarray(embeddings, dtype=np.float32)
    xn = x / np.maximum(np.linalg.norm(x, axis=1, keepdims=True), EPS)
    inv = (1.0 / np.maximum(np.linalg.norm(e, axis=1), EPS)).astype(np.float32)
    return xn, e, inv


def _prep_f16w(xn, e, inv):
    """in_maps for the f16w variant: fp16 transposed normalized shards."""
    xh = np.ascontiguousarray(xn.T).astype(np.float16)
    in_maps = []
    for i in range(CORES):
        lo_r, hi_r = i * N_CORE, (i + 1) * N_CORE
        n_real = max(0, min(hi_r, N_EMB) - lo_r)
        eh = np.zeros((D, N_CORE), dtype=np.float16)
        if n_real > 0:
            sl = e[lo_r:lo_r + n_real] * inv[lo_r:lo_r + n_real][:, None]
            eh[:, :n_real] = sl.T.astype(np.float16)
        in_maps.append({"xh": xh, "eh": eh})
    return in_maps


def _prep_inputs(x, embeddings, variant):
    """Host prep: normalize embeddings, pad, transpose, shard; returns in_maps.

    Works per-core-shard to keep intermediates cache-sized."""
    if variant == "f16w":
        xn, e, inv = _normalize(x, embeddings)
        return _prep_f16w(xn, e, inv)
    if variant in ("f8w", "f8d"):
        xn, e, inv = _normalize(x, embeddings)
        return _prep_f8w(xn, e, inv)
    x = np.asarray(x, dtype=np.float32)
    e = np.asarray(embeddings, dtype=np.float32)
    inv = (1.0 / np.maximum(np.linalg.norm(e, axis=1), EPS)).astype(np.float32)
    xt = np.ascontiguousarray(x.T)               # [D, B]

    in_maps = []
    for i in range(CORES):
        lo_r, hi_r = i * N_CORE, (i + 1) * N_CORE
        n_real = max(0, min(hi_r, N_EMB) - lo_r)
        ent = np.zeros((D, N_CORE), dtype=np.float32)
        if n_real > 0:
            sl = e[lo_r:lo_r + n_real]
            ent[:, :n_real] = sl.T * inv[lo_r:lo_r + n_real][None, :]
        if variant == "f16x3":
            ehi = ent.astype(np.float16)
            elo = (ent - ehi).astype(np.float16)
            in_maps.append({"ehi": ehi, "elo": elo})
        else:
            in_maps.append({"ent": ent})

    if variant == "f16x3":
        xhi = xt.astype(np.float16)
        xlo = (xt - xhi).astype(np.float16)
        for m in in_maps:
            m["xhi"] = xhi
            m["xlo"] = xlo
    else:
        for m in in_maps:
            m["xt"] = xt
    return in_maps


def _merge(results, labels):
    """Host merge: exact global top-10 from per-core per-chunk top-8 pools,
    then the reference's mode computation."""
    vals = np.concatenate([r["vals"] for r in results], axis=1)   # [B, 8*NOUT]
    idx8 = np.concatenate([r["idx"] for r in results], axis=1).astype(np.int64)

    col_base = (np.arange(NOUT, dtype=np.int64) // 8) * CHUNK      # chunk offset
    core_base = np.repeat(np.arange(CORES, dtype=np.int64) * N_CORE, NOUT)
    g = idx8 + np.tile(col_base, CORES)[None, :] + core_base[None, :]

    # padding rows (g >= N_EMB) are zero embeddings: exclude
    u = vals.view(np.uint32)
    key = np.where(u & 0x80000000, ~u, u | 0x80000000).astype(np.uint64)
    combo = ((np.uint64(0xFFFFFFFF) - key) << np.uint64(17)) | g.astype(np.uint64)
    combo[g >= N_EMB] = np.uint64(0xFFFFFFFFFFFFFFFF)
    order = np.argsort(combo, axis=1, kind="stable")[:, :K_NEIGH]
    neighbors = np.take_along_axis(g, order, axis=1)               # [B, 10]

    labels = np.asarray(labels)
    nl = labels[neighbors].astype(np.int64)                        # [B, 10]
    eq = nl[:, :, None] == nl[:, None, :]
    counts = eq.sum(-1)
    mkey = counts * (NUM_CLASSES + 1) + (NUM_CLASSES - nl)
    mi = np.argmax(mkey, axis=1)
    pred = np.take_along_axis(nl, mi[:, None], axis=1)[:, 0]
    return pred.astype(labels.dtype)


class _Runner:
    """Caches the shard_map-jitted executable across calls (mirrors
    bass2jax.run_bass_via_pjrt's multi-core path, which re-traces per call)."""

    def __init__(self, variant):
        import jax
        import concourse.mybir as mb
        from concourse import bass2jax
        from jax.experimental.shard_map import shard_map
        from jax.sharding import Mesh, PartitionSpec

        bass2jax.install_neuronx_cc_hook()
        self.jax = jax
        nc = _get_nc(variant)
        partition_name = (nc.partition_id_tensor.name
                          if nc.partition_id_tensor else None)
        in_names, out_names, out_avals, zeros = [], [], [], []
        for alloc in nc.m.functions[0].allocations:
            if not isinstance(alloc, mb.MemoryLocationSet):
                continue
            name = alloc.memorylocations[0].name
            if alloc.kind == "ExternalInput":
                if name != partition_name:
                    in_names.append(name)
            elif alloc.kind == "ExternalOutput":
                shape = tuple(alloc.tensor_shape)
                dtype = mb.dt.np(alloc.dtype)
                out_avals.append(jax.core.ShapedArray(shape, dtype))
                out_names.append(name)
                zeros.append(np.zeros((CORES * shape[0],) + shape[1:], dtype))
        self.in_names = list(in_names)
        self.out_names = out_names
        self.out_avals = out_avals
        self.zeros = zeros
        n_params = len(in_names)
        all_names = in_names + out_names
        if partition_name is not None:
            all_names = all_names + [partition_name]
        donate = tuple(range(n_params, n_params + len(out_names)))

        def _body(*args):
            operands = list(args)
            if partition_name is not None:
                operands.append(bass2jax.partition_id_tensor())
            outs = bass2jax._bass_exec_p.bind(
                *operands,
                out_avals=tuple(out_avals),
                in_names=tuple(all_names),
                out_names=tuple(out_names),
                lowering_input_output_aliases=(),
                sim_require_finite=True,
                sim_require_nnan=True,
                nc=nc,
            )
            return tuple(outs)

        devices = jax.devices()[:CORES]
        self.mesh = Mesh(np.asarray(devices), ("core",))
        self.pspec = PartitionSpec("core")
        in_specs = (self.pspec,) * (n_params + len(out_names))
        out_specs = (self.pspec,) * len(out_names)
        self.sharded = jax.jit(
            shard_map(_body, mesh=self.mesh, in_specs=in_specs,
                      out_specs=out_specs, check_rep=False),
            donate_argnums=donate, keep_unused=True,
        )

    def concat_inputs(self, in_maps):
        return [
            np.concatenate([np.asarray(m[name]) for m in in_maps], axis=0)
            for name in self.in_names
        ]

    def device_put(self, concat_in):
        from jax.sharding import NamedSharding
        sh = NamedSharding(self.mesh, self.pspec)
        return [self.jax.device_put(a, sh) for a in concat_in]

    def execute(self, concat_in):
        zeros = [np.zeros_like(z) for z in self.zeros]
        out_arrs = self.sharded(*concat_in, *zeros)
        return out_arrs

    def run(self, in_maps):
        out_arrs = self.execute(self.concat_inputs(in_maps))
        return [
            {
                name: np.asarray(out_arrs[i]).reshape(
                    CORES, *self.out_avals[i].shape)[c]
                for i, name in enumerate(self.out_names)
            }
            for c in range(CORES)
        ]


_RUNNERS = {}


def _get_runner(variant=None):
    variant = variant or MM_DTYPE
    if variant not in _RUNNERS:
        _RUNNERS[variant] = _Runner(variant)
    return _RUNNERS[variant]


def _mode_pred(neighbors, labels):
    """Reference's torch.mode semantics on gathered neighbor labels."""
    labels = np.asarray(labels)
    nl = labels[neighbors].astype(np.int64)                        # [B, 10]
    eq = nl[:, :, None] == nl[:, None, :]
    counts = eq.sum(-1)
    mkey = counts * (NUM_CLASSES + 1) + (NUM_CLASSES - nl)
    mi = np.argmax(mkey, axis=1)
    pred = np.take_along_axis(nl, mi[:, None], axis=1)[:, 0]
    return pred.astype(labels.dtype)


def _merge_f16w(results, labels, xn, e, inv, margin=MARGIN):
    """Select windows >= (10th-best window max) - margin, rescore those
    candidates exactly in fp64, exact global top-10, then mode."""
    wv = np.stack([r["wvals"] for r in results], axis=1)      # [B, 8, 32]
    wi = np.stack([r["widx"] for r in results], axis=1).astype(np.int64)
    wi[:, :, NSEL:] += HALF_A   # half-B indices are relative to its slice
    gw = wi + (np.arange(CORES, dtype=np.int64) * WPC)[None, :, None]
    wv = wv.reshape(B, CORES * 2 * NSEL)
    gw = gw.reshape(B, CORES * 2 * NSEL)

    w10 = np.partition(wv, wv.shape[1] - K_NEIGH, axis=1)[:, wv.shape[1] - K_NEIGH]
    keep = wv >= (w10[:, None] - margin)
    smax = int(keep.sum(axis=1).max())

    # top-smax windows per row by value; mask out ones below the cutoff
    order = np.argsort(-wv, axis=1, kind="stable")[:, :smax]
    sel_g = np.take_along_axis(gw, order, axis=1)              # [B, smax]
    sel_keep = np.take_along_axis(keep, order, axis=1)

    # rescore grouped by window: each window's embeddings are one contiguous
    # 32-row slice, shared by every query that selected it (~6400 windows
    # total vs ~170k (row, window) pairs -> tiny gathers, BLAS-sized GEMMs)
    e = np.asarray(e, dtype=np.float32)
    xn32 = np.ascontiguousarray(xn, dtype=np.float32)
    rows_idx, slots = np.nonzero(sel_keep)
    wins = sel_g[rows_idx, slots]
    order = np.argsort(wins, kind="stable")
    rows_idx, slots, wins = rows_idx[order], slots[order], wins[order]
    uniq, starts = np.unique(wins, return_index=True)
    bounds = np.append(starts, len(wins))

    sims = np.full((B, smax, WWIN), -np.inf, dtype=np.float32)
    for ui in range(len(uniq)):
        w = int(uniq[ui])
        c0, c1 = w * WWIN, min(w * WWIN + WWIN, N_EMB)
        if c1 <= c0:
            continue
        s0, s1 = bounds[ui], bounds[ui + 1]
        en_w = e[c0:c1] * inv[c0:c1][:, None]                  # [<=32, D]
        sblk = xn32[rows_idx[s0:s1]] @ en_w.T                  # [nrows, <=32]
        sims[rows_idx[s0:s1], slots[s0:s1], :c1 - c0] = sblk

    cand = (sel_g[:, :, None] * WWIN +
            np.arange(WWIN, dtype=np.int64)[None, None, :]).reshape(B, -1)
    sims = sims.reshape(B, -1)

    # exact top-10 by (-sim, cand) via an order-preserving uint64 key
    u = sims.view(np.uint32)
    mono = np.where(u & 0x80000000, ~u, u | 0x80000000).astype(np.uint64)
    combo = ((np.uint64(0xFFFFFFFF) - mono) << np.uint64(17)) | \
        cand.astype(np.uint64)
    combo[sims == -np.inf] = np.uint64(0xFFFFFFFFFFFFFFFF)
    ordr = np.argsort(combo, axis=1, kind="stable")[:, :K_NEIGH]
    neighbors = np.take_along_axis(cand, ordr, axis=1)
    return _mode_pred(neighbors, labels)


def _merge_f8d(results, labels, xn, e, inv, margin):
    """Host-side window selection from the full per-window-max arrays, then
    the window-grouped exact rescore."""
    wv = np.concatenate([r["wmax"] for r in results], axis=1)   # [B, 8*WPC]
    nw = wv.shape[1]
    w10 = np.partition(wv, nw - K_NEIGH, axis=1)[:, nw - K_NEIGH]
    keep = wv >= (w10[:, None] - margin)                        # [B, 8*WPC]

    rows_idx, wins = np.nonzero(keep)        # wins are global window ids
    slots = (np.cumsum(keep, axis=1) - 1)[rows_idx, wins]
    smax = int(keep.sum(axis=1).max())

    e = np.asarray(e, dtype=np.float32)
    xn32 = np.ascontiguousarray(xn, dtype=np.float32)
    order = np.argsort(wins, kind="stable")
    rows_s, slots_s, wins_s = rows_idx[order], slots[order], wins[order]
    uniq, starts = np.unique(wins_s, return_index=True)
    bounds = np.append(starts, len(wins_s))

    sims = np.full((B, smax, WWIN), -np.inf, dtype=np.float32)
    wfull = np.zeros((B, smax), dtype=np.int64)
    wfull[rows_idx, slots] = wins
    for ui in range(len(uniq)):
        w = int(uniq[ui])
        c0, c1 = w * WWIN, min(w * WWIN + WWIN, N_EMB)
        if c1 <= c0:
            continue
        s0, s1 = bounds[ui], bounds[ui + 1]
        en_w = e[c0:c1] * inv[c0:c1][:, None]
        sblk = xn32[rows_s[s0:s1]] @ en_w.T
        sims[rows_s[s0:s1], slots_s[s0:s1], :c1 - c0] = sblk

    cand = (wfull[:, :, None] * WWIN +
            np.arange(WWIN, dtype=np.int64)[None, None, :]).reshape(B, -1)
    sims = sims.reshape(B, -1)
    u = sims.view(np.uint32)
    mono = np.where(u & 0x80000000, ~u, u | 0x80000000).astype(np.uint64)
    combo = ((np.uint64(0xFFFFFFFF) - mono) << np.uint64(17)) | \
        cand.astype(np.uint64)
    combo[sims == -np.inf] = np.uint64(0xFFFFFFFFFFFFFFFF)
    ordr = np.argsort(combo, axis=1, kind="stable")[:, :K_NEIGH]
    neighbors = np.take_along_axis(cand, ordr, axis=1)
    return _mode_pred(neighbors, labels)


def run_on_hw(x, embeddings, variant=None):
    runner = _get_runner(variant)
    in_maps = _prep_inputs(x, embeddings, variant or MM_DTYPE)
    return runner.run(in_maps)


def kernel(x, embeddings, labels):
    variant = MM_DTYPE
    if variant == "f16w":
        xn, e, inv = _normalize(x, embeddings)
        runner = _get_runner(variant)
        results = runner.run(_prep_f16w(xn, e, inv))
        return _merge_f16w(results, labels, xn, e, inv)
    if variant == "f8w":
        xn, e, inv = _normalize(x, embeddings)
        runner = _get_runner(variant)
        results = runner.run(_prep_f8w(xn, e, inv))
        return _merge_f16w(results, labels, xn, e, inv,
                           margin=MARGIN_F8 * F8_SCALE * F8_SCALE)
    if variant == "f8d":
        xn, e, inv = _normalize(x, embeddings)
        runner = _get_runner(variant)
        results = runner.run(_prep_f8w(xn, e, inv))
        return _merge_f8d(results, labels, xn, e, inv,
                          margin=MARGIN_F8 * F8_SCALE * F8_SCALE)
    results = run_on_hw(x, embeddings)
    return _merge(results, labels)


# revision 46
# speedup vs baseline: 1.2536x; 1.0148x over previous
"""Trainium2 Bass kernel for BaselineKNNModel (cosine-sim KNN classifier).

Contract: kernel(**inputs) takes FULL inputs (x [2048,512] f32,
embeddings [100000,512] f32, labels [100000] int) and returns the FULL
output (pred [2048] labels.dtype), distributing work across 8 NeuronCores.

Strategy (database-parallel, per sharding hint):
 - Host: normalize embeddings (cosine denominator), pad N 100000->102400,
   transpose to [512, N]; shard along N across 8 cores (12800 each).
   x normalization is skipped: per-query positive scaling cannot change
   that query's top-k ranking.
 - Device (SPMD, per core): sim tile [128 q, 512 c] = xT.T @ enT chunk via
   PE accumulation over K=512; per tile, VectorE max/max_index extract the
   top-8 values + indices of each 512-candidate chunk (global top-10 of a
   row is contained in the union of its per-chunk top-8s unless >=9 of the
   top-10 fall in one 512-chunk: P ~ 1e-11).
 - Host: merge 8 cores x 25 chunks x top-8 = 1600 candidates/query, exact
   top-10 by (value desc, index asc) = jax.lax.top_k tie order, then the
   reference's mode computation.
"""
import sys

for _p in ("/opt/trn_rl_repo", "/root/.axon_site/_ro/trn_rl_repo"):
    if _p not in sys.path:
        sys.path.insert(0, _p)

import numpy as np

import concourse.bacc as bacc
import concourse.mybir as mybir
import concourse.tile as tile
from concourse import bass_utils

F32 = mybir.dt.float32
F32R = mybir.dt.float32r
F16 = mybir.dt.float16
U32 = mybir.dt.uint32
Copy = mybir.ActivationFunctionType.Copy

B = 2048            # queries
D = 512             # embedding dim
N_EMB = 100000      # database size
K_NEIGH = 10
NUM_CLASSES = 1000
EPS = 1e-8

CORES = 8
N_PAD = 102400      # padded database size (8 * 12800)
N_CORE = N_PAD // CORES     # 12800 candidates per core
CHUNK = 512                 # candidates per sim tile (one PSUM bank)
NCHUNK = N_CORE // CHUNK    # 25
QT = B // 128               # 16 query tiles
KT = D // 128               # 4 k-tiles
NOUT = NCHUNK * 8           # 200 output slots per query per core

# f16w variant: window-max + device window top-16 + host exact rescore
WWIN = 32                   # candidates per window
WPC = N_CORE // WWIN        # 400 windows per core
BIGCHUNK = 1024             # candidates per PSUM tile (2 banks)
NSEL = 16                   # windows kept per (query, core, half)
HALF_A = (7 * BIGCHUNK) // WWIN  # windows in selection half A (224)
MARGIN = 4e-3               # fp16-sim error margin on unit-normalized sims
                            # (measured max |fp16 sim err| ~6e-5, ~60x safety)

# f8w variant: same as f16w but fp8e4m3 DoubleRow matmuls (2 fp8 weights per
# PE cell, K=256 per matmul). Inputs are scaled by F8_SCALE before rounding
# to fp8, so device sims (and window maxes) are scaled by F8_SCALE^2.
F8_SCALE = 16.0
MARGIN_F8 = 2.5e-2          # fp8 margin on unit-normalized sims
                            # (measured max err 7.1e-3 on a sample, rms 1.6e-3)

MM_DTYPE = "f8d"            # "f32" | "f32r" | "f16x3" | "f16w" | "f8w" | "f8d"

_CACHE = {}


def _build(variant):
    """Build + compile the per-core Bass program. Same program on all cores;
    only the `ent*` input shards differ."""
    nc = bacc.Bacc("TRN2", target_bir_lowering=False, debug=False)

    if variant == "noop":  # minimal program for RPC-overhead baselining
        d_nin = nc.dram_tensor("nin", [128, 128], F32, kind="ExternalInput")
        d_nout = nc.dram_tensor("nout", [128, 128], F32, kind="ExternalOutput")
        with tile.TileContext(nc) as tc:
            with tc.tile_pool(name="np0", bufs=1) as pool:
                t = pool.tile([128, 128], F32, tag="t")
                nc.sync.dma_start(t[:, :], d_nin[:, :])
                nc.sync.dma_start(d_nout[:, :], t[:, :])
        nc.compile()
        return nc

    if variant == "f16w":
        return _build_f16w(nc)
    if variant == "f8w":
        return _build_f8w(nc)
    if variant == "f8d":
        return _build_f8d(nc)
    if variant == "f8e":
        return _build_f8e(nc)

    f16 = variant == "f16x3"
    if f16:
        d_xhi = nc.dram_tensor("xhi", [D, B], F16, kind="ExternalInput")
        d_xlo = nc.dram_tensor("xlo", [D, B], F16, kind="ExternalInput")
        d_ehi = nc.dram_tensor("ehi", [D, N_CORE], F16, kind="ExternalInput")
        d_elo = nc.dram_tensor("elo", [D, N_CORE], F16, kind="ExternalInput")
    else:
        in_dt = F32R if variant == "f32r" else F32
        d_xt = nc.dram_tensor("xt", [D, B], in_dt, kind="ExternalInput")
        d_ent = nc.dram_tensor("ent", [D, N_CORE], in_dt, kind="ExternalInput")

    d_vals = nc.dram_tensor("vals", [B, NOUT], F32, kind="ExternalOutput")
    d_idx = nc.dram_tensor("idx", [B, NOUT], U32, kind="ExternalOutput")

    with tile.TileContext(nc) as tc:
        with (
            tc.tile_pool(name="xpool", bufs=1) as xpool,
            tc.tile_pool(name="epool", bufs=3) as epool,
            tc.tile_pool(name="ps", bufs=6, space="PSUM") as ps_pool,
            tc.tile_pool(name="sim", bufs=6) as sim_pool,
            tc.tile_pool(name="acc", bufs=1) as acc_pool,
        ):
            # resident x (stationary operand), k-tiles side by side
            if f16:
                xhi_sb = xpool.tile([128, KT * B], F16, tag="xhi")
                xlo_sb = xpool.tile([128, KT * B], F16, tag="xlo")
                for k in range(KT):
                    nc.sync.dma_start(xhi_sb[:, k * B:(k + 1) * B],
                                      d_xhi[k * 128:(k + 1) * 128, :])
                    nc.sync.dma_start(xlo_sb[:, k * B:(k + 1) * B],
                                      d_xlo[k * 128:(k + 1) * 128, :])
            else:
                xt_sb = xpool.tile([128, KT * B], in_dt, tag="xt")
                for k in range(KT):
                    nc.sync.dma_start(xt_sb[:, k * B:(k + 1) * B],
                                      d_xt[k * 128:(k + 1) * 128, :])

            # result accumulators, [128, QT*NOUT], column q*NOUT + c*8 + j
            vals_sb = acc_pool.tile([128, QT * NOUT], F32, tag="vacc")
            idx_sb = acc_pool.tile([128, QT * NOUT], U32, tag="iacc")

            for c in range(NCHUNK):
                c0 = c * CHUNK
                if f16:
                    ehi_sb = epool.tile([128, KT * CHUNK], F16, tag="ehi")
                    elo_sb = epool.tile([128, KT * CHUNK], F16, tag="elo")
                    for k in range(KT):
                        nc.sync.dma_start(ehi_sb[:, k * CHUNK:(k + 1) * CHUNK],
                                          d_ehi[k * 128:(k + 1) * 128, c0:c0 + CHUNK])
                        nc.sync.dma_start(elo_sb[:, k * CHUNK:(k + 1) * CHUNK],
                                          d_elo[k * 128:(k + 1) * 128, c0:c0 + CHUNK])
                else:
                    en_sb = epool.tile([128, KT * CHUNK], in_dt, tag="en")
                    for k in range(KT):
                        nc.sync.dma_start(en_sb[:, k * CHUNK:(k + 1) * CHUNK],
                                          d_ent[k * 128:(k + 1) * 128, c0:c0 + CHUNK])

                for q in range(QT):
                    ps = ps_pool.tile([128, CHUNK], F32, tag="ps")
                    if variant == "f16x3":
                        nmm = 3 * KT
                        i = 0
                        for k in range(KT):
                            xh = xhi_sb[:, k * B + q * 128: k * B + (q + 1) * 128]
                            xl = xlo_sb[:, k * B + q * 128: k * B + (q + 1) * 128]
                            eh = ehi_sb[:, k * CHUNK:(k + 1) * CHUNK]
                            el = elo_sb[:, k * CHUNK:(k + 1) * CHUNK]
                            for (a, bb) in ((xh, eh), (xh, el), (xl, eh)):
                                nc.tensor.matmul(ps[:, :], a, bb,
                                                 start=(i == 0), stop=(i == nmm - 1))
                                i += 1
                    else:
                        for k in range(KT):
                            lhsT = xt_sb[:, k * B + q * 128: k * B + (q + 1) * 128]
                            rhs = en_sb[:, k * CHUNK:(k + 1) * CHUNK]
                            nc.tensor.matmul(ps[:, :], lhsT, rhs,
                                             start=(k == 0), stop=(k == KT - 1))

                    sim = sim_pool.tile([128, CHUNK], F32, tag="sim")
                    nc.scalar.activation(sim[:, :], ps[:, :], Copy)

                    o = q * NOUT + c * 8
                    nc.vector.max(vals_sb[:, o:o + 8], sim[:, :])
                    nc.vector.max_index(idx_sb[:, o:o + 8], vals_sb[:, o:o + 8],
                                        sim[:, :])

            for q in range(QT):
                nc.sync.dma_start(d_vals[q * 128:(q + 1) * 128, :],
                                  vals_sb[:, q * NOUT:(q + 1) * NOUT])
                nc.sync.dma_start(d_idx[q * 128:(q + 1) * 128, :],
                                  idx_sb[:, q * NOUT:(q + 1) * NOUT])

    nc.compile()
    return nc


def _build_f16w(nc):
    """fp16 single-pass matmul; per-tile 16-wide window max (DVE reduce,
    PSUM-direct); per-core-half top-16 windows per query via
    max/match_replace (first half's selection overlaps the main loop);
    host rescores the selected windows exactly."""
    Max = mybir.AluOpType.max
    X = mybir.AxisListType.X

    d_xh = nc.dram_tensor("xh", [D, B], F16, kind="ExternalInput")
    d_eh = nc.dram_tensor("eh", [D, N_CORE], F16, kind="ExternalInput")
    d_wvals = nc.dram_tensor("wvals", [B, 2 * NSEL], F32, kind="ExternalOutput")
    d_widx = nc.dram_tensor("widx", [B, 2 * NSEL], U32, kind="ExternalOutput")

    # chunk layout: 12 x 1024 + 1 x 512 = 12800
    chunks = [(i * BIGCHUNK, BIGCHUNK) for i in range(N_CORE // BIGCHUNK)]
    rem = N_CORE - (N_CORE // BIGCHUNK) * BIGCHUNK
    if rem:
        chunks.append((N_CORE - rem, rem))
    # selection halves aligned to chunk boundaries:
    # half A = chunks 0-6 (448 windows), half B = chunks 7-12 (352 windows)
    HALF_B = WPC - HALF_A

    def select(wq, vout, iout, o, width, mr_pool):
        nc.vector.max(vout[:, o:o + 8], wq)
        nc.vector.max_index(iout[:, o:o + 8], vout[:, o:o + 8], wq)
        mr = mr_pool.tile([128, width], F32, tag="mr")
        nc.vector.match_replace(mr[:, :width], vout[:, o:o + 8], wq, -1e30)
        nc.vector.max(vout[:, o + 8:o + 16], mr[:, :width])
        nc.vector.max_index(iout[:, o + 8:o + 16],
                            vout[:, o + 8:o + 16], mr[:, :width])

    with tile.TileContext(nc) as tc:
        with (
            tc.tile_pool(name="xpool", bufs=1) as xpool,
            tc.tile_pool(name="epool", bufs=3) as epool,
            tc.tile_pool(name="ps", bufs=3, space="PSUM") as ps_pool,
            tc.tile_pool(name="wacc", bufs=1) as wacc_pool,
            tc.tile_pool(name="mrp", bufs=4) as mr_pool,
            tc.tile_pool(name="outp", bufs=1) as out_pool,
        ):
            xh_sb = xpool.tile([128, KT * B], F16, tag="xh")
            for k in range(KT):
                nc.sync.dma_start(xh_sb[:, k * B:(k + 1) * B],
                                  d_xh[k * 128:(k + 1) * 128, :])

            wmax_sb = wacc_pool.tile([128, QT * WPC], F32, tag="wacc")
            vout_sb = out_pool.tile([128, QT * 2 * NSEL], F32, tag="vout")
            iout_sb = out_pool.tile([128, QT * 2 * NSEL], U32, tag="iout")

            for ci, (c0, cw) in enumerate(chunks):
                eh_sb = epool.tile([128, KT * BIGCHUNK], F16, tag="eh")
                for k in range(KT):
                    nc.sync.dma_start(eh_sb[:, k * cw:(k + 1) * cw],
                                      d_eh[k * 128:(k + 1) * 128, c0:c0 + cw])
                for q in range(QT):
                    ps = ps_pool.tile([128, BIGCHUNK], F32, tag="ps")
                    for s in range(cw // 512):
                        for k in range(KT):
                            nc.tensor.matmul(
                                ps[:, s * 512:(s + 1) * 512],
                                xh_sb[:, k * B + q * 128: k * B + (q + 1) * 128],
                                eh_sb[:, k * cw + s * 512: k * cw + s * 512 + 512],
                                start=(k == 0), stop=(k == KT - 1))
                    nwin = cw // WWIN
                    wslot = q * WPC + c0 // WWIN
                    nc.vector.tensor_reduce(
                        wmax_sb[:, wslot:wslot + nwin],
                        ps[:, :cw].rearrange("p (w i) -> p w i", i=WWIN),
                        axis=X, op=Max)
                # half A (windows [0, HALF_A)) is complete after chunk 6;
                # spread its per-q selection over chunks 6..12 (2-3 q each)
                if ci >= 6:
                    n_grp = len(chunks) - 6
                    qs = [q for q in range(QT) if q % n_grp == ci - 6]
                    for q in qs:
                        select(wmax_sb[:, q * WPC:q * WPC + HALF_A],
                               vout_sb, iout_sb, q * 2 * NSEL, HALF_A, mr_pool)

            for q in range(QT):  # half B (windows [HALF_A, WPC))
                select(wmax_sb[:, q * WPC + HALF_A:(q + 1) * WPC],
                       vout_sb, iout_sb, q * 2 * NSEL + NSEL, HALF_B, mr_pool)

            for q in range(QT):
                nc.sync.dma_start(d_wvals[q * 128:(q + 1) * 128, :],
                                  vout_sb[:, q * 2 * NSEL:(q + 1) * 2 * NSEL])
                nc.sync.dma_start(d_widx[q * 128:(q + 1) * 128, :],
                                  iout_sb[:, q * 2 * NSEL:(q + 1) * 2 * NSEL])

    nc.compile()
    return nc


def _build_f8w(nc):
    """Same structure as f16w, but fp8e4m3 DoubleRow matmuls: operands carry
    [partition, j(2), cols] APs; each matmul contracts 256 dims (2 k-groups
    of 128), so K=512 takes 2 matmuls per 512-wide output slice."""
    Max = mybir.AluOpType.max
    X = mybir.AxisListType.X
    F8 = mybir.dt.float8e4
    DR = mybir.MatmulPerfMode.DoubleRow

    d_x8 = nc.dram_tensor("x8", [D, B], F8, kind="ExternalInput")
    d_e8 = nc.dram_tensor("e8", [D, N_CORE], F8, kind="ExternalInput")
    d_wvals = nc.dram_tensor("wvals", [B, 2 * NSEL], F32, kind="ExternalOutput")
    d_widx = nc.dram_tensor("widx", [B, 2 * NSEL], U32, kind="ExternalOutput")

    chunks = [(i * BIGCHUNK, BIGCHUNK) for i in range(N_CORE // BIGCHUNK)]
    rem = N_CORE - (N_CORE // BIGCHUNK) * BIGCHUNK
    if rem:
        chunks.append((N_CORE - rem, rem))
    HALF_B = WPC - HALF_A

    def select(wq, vout, iout, o, width, mr_pool):
        nc.vector.max(vout[:, o:o + 8], wq)
        nc.vector.max_index(iout[:, o:o + 8], vout[:, o:o + 8], wq)
        mr = mr_pool.tile([128, width], F32, tag="mr")
        nc.vector.match_replace(mr[:, :width], vout[:, o:o + 8], wq, -1e30)
        nc.vector.max(vout[:, o + 8:o + 16], mr[:, :width])
        nc.vector.max_index(iout[:, o + 8:o + 16],
                            vout[:, o + 8:o + 16], mr[:, :width])

    with tile.TileContext(nc) as tc:
        with (
            tc.tile_pool(name="xpool", bufs=1) as xpool,
            tc.tile_pool(name="epool", bufs=3) as epool,
            tc.tile_pool(name="ps", bufs=3, space="PSUM") as ps_pool,
            tc.tile_pool(name="wacc", bufs=1) as wacc_pool,
            tc.tile_pool(name="mrp", bufs=4) as mr_pool,
            tc.tile_pool(name="outp", bufs=1) as out_pool,
        ):
            # [g][j][cols] layout: row-range g*256 + j*128 of the [D, *] input
            x_sb = xpool.tile([128, 4 * B], F8, tag="x8")
            for g in range(2):
                for j in range(2):
                    r0 = g * 256 + j * 128
                    nc.sync.dma_start(x_sb[:, (g * 2 + j) * B:(g * 2 + j + 1) * B],
                                      d_x8[r0:r0 + 128, :])

            wmax_sb = wacc_pool.tile([128, QT * WPC], F32, tag="wacc")
            vout_sb = out_pool.tile([128, QT * 2 * NSEL], F32, tag="vout")
            iout_sb = out_pool.tile([128, QT * 2 * NSEL], U32, tag="iout")

            for ci, (c0, cw) in enumerate(chunks):
                eh_sb = epool.tile([128, 4 * BIGCHUNK], F8, tag="e8")
                for g in range(2):
                    for j in range(2):
                        r0 = g * 256 + j * 128
                        nc.sync.dma_start(
                            eh_sb[:, (g * 2 + j) * cw:(g * 2 + j + 1) * cw],
                            d_e8[r0:r0 + 128, c0:c0 + cw])
                for q in range(QT):
                    ps = ps_pool.tile([128, BIGCHUNK], F32, tag="ps")
                    for s in range(cw // 512):
                        for g in range(2):
                            lhsT = x_sb[:, g * 2 * B:(g + 1) * 2 * B].rearrange(
                                "p (j b) -> p j b", j=2)[:, :, q * 128:(q + 1) * 128]
                            rhs = eh_sb[:, g * 2 * cw:(g + 1) * 2 * cw].rearrange(
                                "p (j n) -> p j n", j=2)[:, :, s * 512:(s + 1) * 512]
                            nc.tensor.matmul(ps[:, s * 512:(s + 1) * 512],
                                             lhsT, rhs, perf_mode=DR,
                                             start=(g == 0), stop=(g == 1))
                    nwin = cw // WWIN
                    wslot = q * WPC + c0 // WWIN
                    nc.vector.tensor_reduce(
                        wmax_sb[:, wslot:wslot + nwin],
                        ps[:, :cw].rearrange("p (w i) -> p w i", i=WWIN),
                        axis=X, op=Max)
                if ci >= 6:
                    n_grp = len(chunks) - 6
                    qs = [q for q in range(QT) if q % n_grp == ci - 6]
                    for q in qs:
                        select(wmax_sb[:, q * WPC:q * WPC + HALF_A],
                               vout_sb, iout_sb, q * 2 * NSEL, HALF_A, mr_pool)

            for q in range(QT):
                select(wmax_sb[:, q * WPC + HALF_A:(q + 1) * WPC],
                       vout_sb, iout_sb, q * 2 * NSEL + NSEL, HALF_B, mr_pool)

            for q in range(QT):
                nc.sync.dma_start(d_wvals[q * 128:(q + 1) * 128, :],
                                  vout_sb[:, q * 2 * NSEL:(q + 1) * 2 * NSEL])
                nc.sync.dma_start(d_widx[q * 128:(q + 1) * 128, :],
                                  iout_sb[:, q * 2 * NSEL:(q + 1) * 2 * NSEL])

    nc.compile()
    return nc


_F8_LUT = None


def _to_f8(a):
    """Fast float->fp8e4m3: fp16 hardware cast, then a 64K-entry LUT over the
    fp16 bit patterns (ml_dtypes' elementwise astype is ~50x slower). The
    double rounding vs a direct fp32->fp8 cast is harmless here: any
    consistent rounding is covered by the selection margin."""
    global _F8_LUT
    import ml_dtypes
    if _F8_LUT is None:
        with np.errstate(all="ignore"):
            all16 = np.arange(65536, dtype=np.uint16).view(np.float16)
            _F8_LUT = (all16.astype(np.float32)
                       .astype(ml_dtypes.float8_e4m3).view(np.uint8))
    h = a.astype(np.float16).view(np.uint16)
    return _F8_LUT[h].view(ml_dtypes.float8_e4m3)


def _build_f8d(nc):
    """f8w minus on-device window selection: the full per-window max array
    ships to the host (3.3MB/core), which does the margin selection itself.
    ScalarE stages PSUM->SBUF so the DVE reduce pays the SBUF (not PSUM)
    access bubble; DVE runs nothing but the 208 window-max reduces."""
    Max = mybir.AluOpType.max
    X = mybir.AxisListType.X
    F8 = mybir.dt.float8e4
    DR = mybir.MatmulPerfMode.DoubleRow
    Copy = mybir.ActivationFunctionType.Copy

    d_x8 = nc.dram_tensor("x8", [D, B], F8, kind="ExternalInput")
    d_e8 = nc.dram_tensor("e8", [D, N_CORE], F8, kind="ExternalInput")
    d_wmax = nc.dram_tensor("wmax", [B, WPC], F32, kind="ExternalOutput")

    chunks = [(i * BIGCHUNK, BIGCHUNK) for i in range(N_CORE // BIGCHUNK)]
    rem = N_CORE - (N_CORE // BIGCHUNK) * BIGCHUNK
    if rem:
        chunks.append((N_CORE - rem, rem))

    with tile.TileContext(nc) as tc:
        with (
            tc.tile_pool(name="xpool", bufs=1) as xpool,
            tc.tile_pool(name="epool", bufs=3) as epool,
            tc.tile_pool(name="ps", bufs=3, space="PSUM") as ps_pool,
            tc.tile_pool(name="stg", bufs=3) as stg_pool,
            tc.tile_pool(name="wacc", bufs=1) as wacc_pool,
        ):
            x_sb = xpool.tile([128, 4 * B], F8, tag="x8")
            for g in range(2):
                for j in range(2):
                    r0 = g * 256 + j * 128
                    nc.sync.dma_start(x_sb[:, (g * 2 + j) * B:(g * 2 + j + 1) * B],
                                      d_x8[r0:r0 + 128, :])

            wmax_sb = wacc_pool.tile([128, QT * WPC], F32, tag="wacc")

            for (c0, cw) in chunks:
                eh_sb = epool.tile([128, 4 * BIGCHUNK], F8, tag="e8")
                for g in range(2):
                    for j in range(2):
                        r0 = g * 256 + j * 128
                        nc.sync.dma_start(
                            eh_sb[:, (g * 2 + j) * cw:(g * 2 + j + 1) * cw],
                            d_e8[r0:r0 + 128, c0:c0 + cw])
                for q in range(QT):
                    ps = ps_pool.tile([128, BIGCHUNK], F32, tag="ps")
                    for s in range(cw // 512):
                        for g in range(2):
                            lhsT = x_sb[:, g * 2 * B:(g + 1) * 2 * B].rearrange(
                                "p (j b) -> p j b", j=2)[:, :, q * 128:(q + 1) * 128]
                            rhs = eh_sb[:, g * 2 * cw:(g + 1) * 2 * cw].rearrange(
                                "p (j n) -> p j n", j=2)[:, :, s * 512:(s + 1) * 512]
                            nc.tensor.matmul(ps[:, s * 512:(s + 1) * 512],
                                             lhsT, rhs, perf_mode=DR,
                                             start=(g == 0), stop=(g == 1))
                    stg = stg_pool.tile([128, BIGCHUNK], F32, tag="stg")
                    nc.scalar.activation(stg[:, :cw], ps[:, :cw], Copy)
                    nwin = cw // WWIN
                    wslot = q * WPC + c0 // WWIN
                    nc.vector.tensor_reduce(
                        wmax_sb[:, wslot:wslot + nwin],
                        stg[:, :cw].rearrange("p (w i) -> p w i", i=WWIN),
                        axis=X, op=Max)

            for q in range(QT):
                nc.sync.dma_start(d_wmax[q * 128:(q + 1) * 128, :],
                                  wmax_sb[:, q * WPC:(q + 1) * WPC])

    nc.compile()
    return nc


def _build_f8e(nc):
    """f8d with wider DVE reduces (two staged PSUM tiles -> one 2048-wide
    window-max, halving the per-op SBUF bubble count) and per-half early
    wmax DMA-out so the output transfer overlaps the main loop."""
    Max = mybir.AluOpType.max
    X = mybir.AxisListType.X
    F8 = mybir.dt.float8e4
    DR = mybir.MatmulPerfMode.DoubleRow
    Copy = mybir.ActivationFunctionType.Copy

    d_x8 = nc.dram_tensor("x8", [D, B], F8, kind="ExternalInput")
    d_e8 = nc.dram_tensor("e8", [D, N_CORE], F8, kind="ExternalInput")
    d_wmax = nc.dram_tensor("wmax", [B, WPC], F32, kind="ExternalOutput")

    BC = 2048  # 4 PSUM banks per tile; 6x2048 + 1x512 = 12800
    chunks = [(i * BC, BC) for i in range(N_CORE // BC)]
    rem = N_CORE - (N_CORE // BC) * BC
    if rem:
        chunks.append((N_CORE - rem, rem))
    AWIN = (4 * BC) // WWIN  # 256 windows (chunks 0-3) ship mid-loop

    with tile.TileContext(nc) as tc:
        with (
            tc.tile_pool(name="xpool", bufs=1) as xpool,
            tc.tile_pool(name="epool", bufs=3) as epool,
            tc.tile_pool(name="ps", bufs=2, space="PSUM") as ps_pool,
            tc.tile_pool(name="stg", bufs=3) as stg_pool,
            tc.tile_pool(name="wacc", bufs=1) as wacc_pool,
        ):
            x_sb = xpool.tile([128, 4 * B], F8, tag="x8")
            for g in range(2):
                for j in range(2):
                    r0 = g * 256 + j * 128
                    nc.sync.dma_start(x_sb[:, (g * 2 + j) * B:(g * 2 + j + 1) * B],
                                      d_x8[r0:r0 + 128, :])

            wmax_sb = wacc_pool.tile([128, QT * WPC], F32, tag="wacc")

            for ci, (c0, cw) in enumerate(chunks):
                eh_sb = epool.tile([128, 4 * BC], F8, tag="e8")
                for g in range(2):
                    for j in range(2):
                        r0 = g * 256 + j * 128
                        nc.sync.dma_start(
                            eh_sb[:, (g * 2 + j) * cw:(g * 2 + j + 1) * cw],
                            d_e8[r0:r0 + 128, c0:c0 + cw])
                for q in range(QT):
                    ps = ps_pool.tile([128, BC], F32, tag="ps")
                    for s in range(cw // 512):
                        for g in range(2):
                            lhsT = x_sb[:, g * 2 * B:(g + 1) * 2 * B].rearrange(
                                "p (j b) -> p j b", j=2)[:, :, q * 128:(q + 1) * 128]
                            rhs = eh_sb[:, g * 2 * cw:(g + 1) * 2 * cw].rearrange(
                                "p (j n) -> p j n", j=2)[:, :, s * 512:(s + 1) * 512]
                            nc.tensor.matmul(ps[:, s * 512:(s + 1) * 512],
                                             lhsT, rhs, perf_mode=DR,
                                             start=(g == 0), stop=(g == 1))
                    stg = stg_pool.tile([128, BC], F32, tag="stg")
                    nc.scalar.activation(stg[:, :cw], ps[:, :cw], Copy)
                    nwin = cw // WWIN
                    wslot = q * WPC + c0 // WWIN
                    nc.vector.tensor_reduce(
                        wmax_sb[:, wslot:wslot + nwin],
                        stg[:, :cw].rearrange("p (w i) -> p w i", i=WWIN),
                        axis=X, op=Max)
                    if ci == 3:  # chunks 0-3 reduced for q: ship 256 windows
                        nc.sync.dma_start(
                            d_wmax[q * 128:(q + 1) * 128, :AWIN],
                            wmax_sb[:, q * WPC:q * WPC + AWIN])

            for q in range(QT):
                nc.sync.dma_start(d_wmax[q * 128:(q + 1) * 128, AWIN:],
                                  wmax_sb[:, q * WPC + AWIN:(q + 1) * WPC])

    nc.compile()
    return nc


def _prep_f8w(xn, e, inv):
    """in_maps for the f8w variant: fp8e4m3 transposed normalized shards,
    scaled by F8_SCALE to stay clear of the fp8 subnormal range."""
    import ml_dtypes
    f8 = ml_dtypes.float8_e4m3
    x8 = _to_f8(np.ascontiguousarray(xn.T) * np.float32(F8_SCALE))
    in_maps = []
    for i in range(CORES):
        lo_r, hi_r = i * N_CORE, (i + 1) * N_CORE
        n_real = max(0, min(hi_r, N_EMB) - lo_r)
        e8 = np.zeros((D, N_CORE), dtype=f8)
        if n_real > 0:
            sl = e[lo_r:lo_r + n_real] * (inv[lo_r:lo_r + n_real]
                                          * np.float32(F8_SCALE))[:, None]
            e8[:, :n_real] = _to_f8(sl.T)
        in_maps.append({"x8": x8, "e8": e8})
    return in_maps


def _get_nc(variant=None):
    variant = variant or MM_DTYPE
    if variant not in _CACHE:
        _CACHE[variant] = _build(variant)
    return _CACHE[variant]


def _normalize(x, embeddings):
    x = np.asarray(x, dtype=np.float32)
    e = np.asarray(embeddings, dtype=np.float32)
    xn = x / np.maximum(np.linalg.norm(x, axis=1, keepdims=True), EPS)
    inv = (1.0 / np.maximum(np.linalg.norm(e, axis=1), EPS)).astype(np.float32)
    return xn, e, inv


def _prep_f16w(xn, e, inv):
    """in_maps for the f16w variant: fp16 transposed normalized shards."""
    xh = np.ascontiguousarray(xn.T).astype(np.float16)
    in_maps = []
    for i in range(CORES):
        lo_r, hi_r = i * N_CORE, (i + 1) * N_CORE
        n_real = max(0, min(hi_r, N_EMB) - lo_r)
        eh = np.zeros((D, N_CORE), dtype=np.float16)
        if n_real > 0:
            sl = e[lo_r:lo_r + n_real] * inv[lo_r:lo_r + n_real][:, None]
            eh[:, :n_real] = sl.T.astype(np.float16)
        in_maps.append({"xh": xh, "eh": eh})
    return in_maps


def _prep_inputs(x, embeddings, variant):
    """Host prep: normalize embeddings, pad, transpose, shard; returns in_maps.

    Works per-core-shard to keep intermediates cache-sized."""
    if variant == "f16w":
        xn, e, inv = _normalize(x, embeddings)
        return _prep_f16w(xn, e, inv)
    if variant in ("f8w", "f8d", "f8e"):
        xn, e, inv = _normalize(x, embeddings)
        return _prep_f8w(xn, e, inv)
    x = np.asarray(x, dtype=np.float32)
    e = np.asarray(embeddings, dtype=np.float32)
    inv = (1.0 / np.maximum(np.linalg.norm(e, axis=1), EPS)).astype(np.float32)
    xt = np.ascontiguousarray(x.T)               # [D, B]

    in_maps = []
    for i in range(CORES):
        lo_r, hi_r = i * N_CORE, (i + 1) * N_CORE
        n_real = max(0, min(hi_r, N_EMB) - lo_r)
        ent = np.zeros((D, N_CORE), dtype=np.float32)
        if n_real > 0:
            sl = e[lo_r:lo_r + n_real]
            ent[:, :n_real] = sl.T * inv[lo_r:lo_r + n_real][None, :]
        if variant == "f16x3":
            ehi = ent.astype(np.float16)
            elo = (ent - ehi).astype(np.float16)
            in_maps.append({"ehi": ehi, "elo": elo})
        else:
            in_maps.append({"ent": ent})

    if variant == "f16x3":
        xhi = xt.astype(np.float16)
        xlo = (xt - xhi).astype(np.float16)
        for m in in_maps:
            m["xhi"] = xhi
            m["xlo"] = xlo
    else:
        for m in in_maps:
            m["xt"] = xt
    return in_maps


def _merge(results, labels):
    """Host merge: exact global top-10 from per-core per-chunk top-8 pools,
    then the reference's mode computation."""
    vals = np.concatenate([r["vals"] for r in results], axis=1)   # [B, 8*NOUT]
    idx8 = np.concatenate([r["idx"] for r in results], axis=1).astype(np.int64)

    col_base = (np.arange(NOUT, dtype=np.int64) // 8) * CHUNK      # chunk offset
    core_base = np.repeat(np.arange(CORES, dtype=np.int64) * N_CORE, NOUT)
    g = idx8 + np.tile(col_base, CORES)[None, :] + core_base[None, :]

    # padding rows (g >= N_EMB) are zero embeddings: exclude
    u = vals.view(np.uint32)
    key = np.where(u & 0x80000000, ~u, u | 0x80000000).astype(np.uint64)
    combo = ((np.uint64(0xFFFFFFFF) - key) << np.uint64(17)) | g.astype(np.uint64)
    combo[g >= N_EMB] = np.uint64(0xFFFFFFFFFFFFFFFF)
    order = np.argsort(combo, axis=1, kind="stable")[:, :K_NEIGH]
    neighbors = np.take_along_axis(g, order, axis=1)               # [B, 10]

    labels = np.asarray(labels)
    nl = labels[neighbors].astype(np.int64)                        # [B, 10]
    eq = nl[:, :, None] == nl[:, None, :]
    counts = eq.sum(-1)
    mkey = counts * (NUM_CLASSES + 1) + (NUM_CLASSES - nl)
    mi = np.argmax(mkey, axis=1)
    pred = np.take_along_axis(nl, mi[:, None], axis=1)[:, 0]
    return pred.astype(labels.dtype)


class _Runner:
    """Caches the shard_map-jitted executable across calls (mirrors
    bass2jax.run_bass_via_pjrt's multi-core path, which re-traces per call)."""

    def __init__(self, variant):
        import jax
        import concourse.mybir as mb
        from concourse import bass2jax
        from jax.experimental.shard_map import shard_map
        from jax.sharding import Mesh, PartitionSpec

        bass2jax.install_neuronx_cc_hook()
        self.jax = jax
        nc = _get_nc(variant)
        partition_name = (nc.partition_id_tensor.name
                          if nc.partition_id_tensor else None)
        in_names, out_names, out_avals, zeros = [], [], [], []
        for alloc in nc.m.functions[0].allocations:
            if not isinstance(alloc, mb.MemoryLocationSet):
                continue
            name = alloc.memorylocations[0].name
            if alloc.kind == "ExternalInput":
                if name != partition_name:
                    in_names.append(name)
            elif alloc.kind == "ExternalOutput":
                shape = tuple(alloc.tensor_shape)
                dtype = mb.dt.np(alloc.dtype)
                out_avals.append(jax.core.ShapedArray(shape, dtype))
                out_names.append(name)
                zeros.append(np.zeros((CORES * shape[0],) + shape[1:], dtype))
        self.in_names = list(in_names)
        self.out_names = out_names
        self.out_avals = out_avals
        self.zeros = zeros
        n_params = len(in_names)
        all_names = in_names + out_names
        if partition_name is not None:
            all_names = all_names + [partition_name]
        donate = tuple(range(n_params, n_params + len(out_names)))

        def _body(*args):
            operands = list(args)
            if partition_name is not None:
                operands.append(bass2jax.partition_id_tensor())
            outs = bass2jax._bass_exec_p.bind(
                *operands,
                out_avals=tuple(out_avals),
                in_names=tuple(all_names),
                out_names=tuple(out_names),
                lowering_input_output_aliases=(),
                sim_require_finite=True,
                sim_require_nnan=True,
                nc=nc,
            )
            return tuple(outs)

        devices = jax.devices()[:CORES]
        self.mesh = Mesh(np.asarray(devices), ("core",))
        self.pspec = PartitionSpec("core")
        in_specs = (self.pspec,) * (n_params + len(out_names))
        out_specs = (self.pspec,) * len(out_names)
        self.sharded = jax.jit(
            shard_map(_body, mesh=self.mesh, in_specs=in_specs,
                      out_specs=out_specs, check_rep=False),
            donate_argnums=donate, keep_unused=True,
        )

    def concat_inputs(self, in_maps):
        return [
            np.concatenate([np.asarray(m[name]) for m in in_maps], axis=0)
            for name in self.in_names
        ]

    def device_put(self, concat_in):
        from jax.sharding import NamedSharding
        sh = NamedSharding(self.mesh, self.pspec)
        return [self.jax.device_put(a, sh) for a in concat_in]

    def execute(self, concat_in):
        zeros = [np.zeros_like(z) for z in self.zeros]
        out_arrs = self.sharded(*concat_in, *zeros)
        return out_arrs

    def run(self, in_maps):
        out_arrs = self.execute(self.concat_inputs(in_maps))
        return [
            {
                name: np.asarray(out_arrs[i]).reshape(
                    CORES, *self.out_avals[i].shape)[c]
                for i, name in enumerate(self.out_names)
            }
            for c in range(CORES)
        ]


_RUNNERS = {}


def _get_runner(variant=None):
    variant = variant or MM_DTYPE
    if variant not in _RUNNERS:
        _RUNNERS[variant] = _Runner(variant)
    return _RUNNERS[variant]


def _mode_pred(neighbors, labels):
    """Reference's torch.mode semantics on gathered neighbor labels."""
    labels = np.asarray(labels)
    nl = labels[neighbors].astype(np.int64)                        # [B, 10]
    eq = nl[:, :, None] == nl[:, None, :]
    counts = eq.sum(-1)
    mkey = counts * (NUM_CLASSES + 1) + (NUM_CLASSES - nl)
    mi = np.argmax(mkey, axis=1)
    pred = np.take_along_axis(nl, mi[:, None], axis=1)[:, 0]
    return pred.astype(labels.dtype)


def _merge_f16w(results, labels, xn, e, inv, margin=MARGIN):
    """Select windows >= (10th-best window max) - margin, rescore those
    candidates exactly in fp64, exact global top-10, then mode."""
    wv = np.stack([r["wvals"] for r in results], axis=1)      # [B, 8, 32]
    wi = np.stack([r["widx"] for r in results], axis=1).astype(np.int64)
    wi[:, :, NSEL:] += HALF_A   # half-B indices are relative to its slice
    gw = wi + (np.arange(CORES, dtype=np.int64) * WPC)[None, :, None]
    wv = wv.reshape(B, CORES * 2 * NSEL)
    gw = gw.reshape(B, CORES * 2 * NSEL)

    w10 = np.partition(wv, wv.shape[1] - K_NEIGH, axis=1)[:, wv.shape[1] - K_NEIGH]
    keep = wv >= (w10[:, None] - margin)
    smax = int(keep.sum(axis=1).max())

    # top-smax windows per row by value; mask out ones below the cutoff
    order = np.argsort(-wv, axis=1, kind="stable")[:, :smax]
    sel_g = np.take_along_axis(gw, order, axis=1)              # [B, smax]
    sel_keep = np.take_along_axis(keep, order, axis=1)

    # rescore grouped by window: each window's embeddings are one contiguous
    # 32-row slice, shared by every query that selected it (~6400 windows
    # total vs ~170k (row, window) pairs -> tiny gathers, BLAS-sized GEMMs)
    e = np.asarray(e, dtype=np.float32)
    xn32 = np.ascontiguousarray(xn, dtype=np.float32)
    rows_idx, slots = np.nonzero(sel_keep)
    wins = sel_g[rows_idx, slots]
    order = np.argsort(wins, kind="stable")
    rows_idx, slots, wins = rows_idx[order], slots[order], wins[order]
    uniq, starts = np.unique(wins, return_index=True)
    bounds = np.append(starts, len(wins))

    sims = np.full((B, smax, WWIN), -np.inf, dtype=np.float32)
    for ui in range(len(uniq)):
        w = int(uniq[ui])
        c0, c1 = w * WWIN, min(w * WWIN + WWIN, N_EMB)
        if c1 <= c0:
            continue
        s0, s1 = bounds[ui], bounds[ui + 1]
        en_w = e[c0:c1] * inv[c0:c1][:, None]                  # [<=32, D]
        sblk = xn32[rows_idx[s0:s1]] @ en_w.T                  # [nrows, <=32]
        sims[rows_idx[s0:s1], slots[s0:s1], :c1 - c0] = sblk

    cand = (sel_g[:, :, None] * WWIN +
            np.arange(WWIN, dtype=np.int64)[None, None, :]).reshape(B, -1)
    sims = sims.reshape(B, -1)

    # exact top-10 by (-sim, cand) via an order-preserving uint64 key
    u = sims.view(np.uint32)
    mono = np.where(u & 0x80000000, ~u, u | 0x80000000).astype(np.uint64)
    combo = ((np.uint64(0xFFFFFFFF) - mono) << np.uint64(17)) | \
        cand.astype(np.uint64)
    combo[sims == -np.inf] = np.uint64(0xFFFFFFFFFFFFFFFF)
    ordr = np.argsort(combo, axis=1, kind="stable")[:, :K_NEIGH]
    neighbors = np.take_along_axis(cand, ordr, axis=1)
    return _mode_pred(neighbors, labels)


def _merge_f8d(results, labels, xn, e, inv, margin):
    """Host-side window selection from the full per-window-max arrays, then
    the window-grouped exact rescore."""
    wv = np.concatenate([r["wmax"] for r in results], axis=1)   # [B, 8*WPC]
    nw = wv.shape[1]
    w10 = np.partition(wv, nw - K_NEIGH, axis=1)[:, nw - K_NEIGH]
    keep = wv >= (w10[:, None] - margin)                        # [B, 8*WPC]

    rows_idx, wins = np.nonzero(keep)        # wins are global window ids
    slots = (np.cumsum(keep, axis=1) - 1)[rows_idx, wins]
    smax = int(keep.sum(axis=1).max())

    e = np.asarray(e, dtype=np.float32)
    xn32 = np.ascontiguousarray(xn, dtype=np.float32)
    order = np.argsort(wins, kind="stable")
    rows_s, slots_s, wins_s = rows_idx[order], slots[order], wins[order]
    uniq, starts = np.unique(wins_s, return_index=True)
    bounds = np.append(starts, len(wins_s))

    sims = np.full((B, smax, WWIN), -np.inf, dtype=np.float32)
    wfull = np.zeros((B, smax), dtype=np.int64)
    wfull[rows_idx, slots] = wins
    for ui in range(len(uniq)):
        w = int(uniq[ui])
        c0, c1 = w * WWIN, min(w * WWIN + WWIN, N_EMB)
        if c1 <= c0:
            continue
        s0, s1 = bounds[ui], bounds[ui + 1]
        en_w = e[c0:c1] * inv[c0:c1][:, None]
        sblk = xn32[rows_s[s0:s1]] @ en_w.T
        sims[rows_s[s0:s1], slots_s[s0:s1], :c1 - c0] = sblk

    cand = (wfull[:, :, None] * WWIN +
            np.arange(WWIN, dtype=np.int64)[None, None, :]).reshape(B, -1)
    sims = sims.reshape(B, -1)
    u = sims.view(np.uint32)
    mono = np.where(u & 0x80000000, ~u, u | 0x80000000).astype(np.uint64)
    combo = ((np.uint64(0xFFFFFFFF) - mono) << np.uint64(17)) | \
        cand.astype(np.uint64)
    combo[sims == -np.inf] = np.uint64(0xFFFFFFFFFFFFFFFF)
    ordr = np.argsort(combo, axis=1, kind="stable")[:, :K_NEIGH]
    neighbors = np.take_along_axis(cand, ordr, axis=1)
    return _mode_pred(neighbors, labels)


def run_on_hw(x, embeddings, variant=None):
    runner = _get_runner(variant)
    in_maps = _prep_inputs(x, embeddings, variant or MM_DTYPE)
    return runner.run(in_maps)


def kernel(x, embeddings, labels):
    variant = MM_DTYPE
    if variant == "f16w":
        xn, e, inv = _normalize(x, embeddings)
        runner = _get_runner(variant)
        results = runner.run(_prep_f16w(xn, e, inv))
        return _merge_f16w(results, labels, xn, e, inv)
    if variant == "f8w":
        xn, e, inv = _normalize(x, embeddings)
        runner = _get_runner(variant)
        results = runner.run(_prep_f8w(xn, e, inv))
        return _merge_f16w(results, labels, xn, e, inv,
                           margin=MARGIN_F8 * F8_SCALE * F8_SCALE)
    if variant in ("f8d", "f8e"):
        xn, e, inv = _normalize(x, embeddings)
        runner = _get_runner(variant)
        results = runner.run(_prep_f8w(xn, e, inv))
        return _merge_f8d(results, labels, xn, e, inv,
                          margin=MARGIN_F8 * F8_SCALE * F8_SCALE)
    results = run_on_hw(x, embeddings)
    return _merge(results, labels)


# revision 49
# speedup vs baseline: 1.2552x; 1.0013x over previous
"""Trainium2 Bass kernel for BaselineKNNModel (cosine-sim KNN classifier).

Contract: kernel(**inputs) takes FULL inputs (x [2048,512] f32,
embeddings [100000,512] f32, labels [100000] int) and returns the FULL
output (pred [2048] labels.dtype), distributing work across 8 NeuronCores.

Strategy (database-parallel, per sharding hint):
 - Host: normalize embeddings (cosine denominator), pad N 100000->102400,
   transpose to [512, N]; shard along N across 8 cores (12800 each).
   x normalization is skipped: per-query positive scaling cannot change
   that query's top-k ranking.
 - Device (SPMD, per core): sim tile [128 q, 512 c] = xT.T @ enT chunk via
   PE accumulation over K=512; per tile, VectorE max/max_index extract the
   top-8 values + indices of each 512-candidate chunk (global top-10 of a
   row is contained in the union of its per-chunk top-8s unless >=9 of the
   top-10 fall in one 512-chunk: P ~ 1e-11).
 - Host: merge 8 cores x 25 chunks x top-8 = 1600 candidates/query, exact
   top-10 by (value desc, index asc) = jax.lax.top_k tie order, then the
   reference's mode computation.
"""
import sys

for _p in ("/opt/trn_rl_repo", "/root/.axon_site/_ro/trn_rl_repo"):
    if _p not in sys.path:
        sys.path.insert(0, _p)

import numpy as np

import concourse.bacc as bacc
import concourse.mybir as mybir
import concourse.tile as tile
from concourse import bass_utils

F32 = mybir.dt.float32
F32R = mybir.dt.float32r
F16 = mybir.dt.float16
U32 = mybir.dt.uint32
Copy = mybir.ActivationFunctionType.Copy

B = 2048            # queries
D = 512             # embedding dim
N_EMB = 100000      # database size
K_NEIGH = 10
NUM_CLASSES = 1000
EPS = 1e-8

CORES = 8
N_PAD = 100096      # padded database size (8 * 12512; minimal multiple of
                    # CORES*WWIN at or above N_EMB)
N_CORE = N_PAD // CORES     # 12512 candidates per core
CHUNK = 512                 # candidates per sim tile (one PSUM bank)
NCHUNK = N_CORE // CHUNK    # 25
QT = B // 128               # 16 query tiles
KT = D // 128               # 4 k-tiles
NOUT = NCHUNK * 8           # 200 output slots per query per core

# f16w variant: window-max + device window top-16 + host exact rescore
WWIN = 32                   # candidates per window
WPC = N_CORE // WWIN        # 400 windows per core
BIGCHUNK = 1024             # candidates per PSUM tile (2 banks)
NSEL = 16                   # windows kept per (query, core, half)
HALF_A = (7 * BIGCHUNK) // WWIN  # windows in selection half A (224)
MARGIN = 4e-3               # fp16-sim error margin on unit-normalized sims
                            # (measured max |fp16 sim err| ~6e-5, ~60x safety)

# f8w variant: same as f16w but fp8e4m3 DoubleRow matmuls (2 fp8 weights per
# PE cell, K=256 per matmul). Inputs are scaled by F8_SCALE before rounding
# to fp8, so device sims (and window maxes) are scaled by F8_SCALE^2.
F8_SCALE = 16.0
MARGIN_F8 = 2.5e-2          # fp8 margin on unit-normalized sims
                            # (measured max err 7.1e-3 on a sample, rms 1.6e-3)

MM_DTYPE = "f8e"   # "f32" | "f32r" | "f16x3" | "f16w" | "f8w" | "f8d" | "f8e"

_CACHE = {}


def _build(variant):
    """Build + compile the per-core Bass program. Same program on all cores;
    only the `ent*` input shards differ."""
    nc = bacc.Bacc("TRN2", target_bir_lowering=False, debug=False)

    if variant == "noop":  # minimal program for RPC-overhead baselining
        d_nin = nc.dram_tensor("nin", [128, 128], F32, kind="ExternalInput")
        d_nout = nc.dram_tensor("nout", [128, 128], F32, kind="ExternalOutput")
        with tile.TileContext(nc) as tc:
            with tc.tile_pool(name="np0", bufs=1) as pool:
                t = pool.tile([128, 128], F32, tag="t")
                nc.sync.dma_start(t[:, :], d_nin[:, :])
                nc.sync.dma_start(d_nout[:, :], t[:, :])
        nc.compile()
        return nc

    if variant == "f16w":
        return _build_f16w(nc)
    if variant == "f8w":
        return _build_f8w(nc)
    if variant == "f8d":
        return _build_f8d(nc)
    if variant == "f8e":
        return _build_f8e(nc)

    f16 = variant == "f16x3"
    if f16:
        d_xhi = nc.dram_tensor("xhi", [D, B], F16, kind="ExternalInput")
        d_xlo = nc.dram_tensor("xlo", [D, B], F16, kind="ExternalInput")
        d_ehi = nc.dram_tensor("ehi", [D, N_CORE], F16, kind="ExternalInput")
        d_elo = nc.dram_tensor("elo", [D, N_CORE], F16, kind="ExternalInput")
    else:
        in_dt = F32R if variant == "f32r" else F32
        d_xt = nc.dram_tensor("xt", [D, B], in_dt, kind="ExternalInput")
        d_ent = nc.dram_tensor("ent", [D, N_CORE], in_dt, kind="ExternalInput")

    d_vals = nc.dram_tensor("vals", [B, NOUT], F32, kind="ExternalOutput")
    d_idx = nc.dram_tensor("idx", [B, NOUT], U32, kind="ExternalOutput")

    with tile.TileContext(nc) as tc:
        with (
            tc.tile_pool(name="xpool", bufs=1) as xpool,
            tc.tile_pool(name="epool", bufs=3) as epool,
            tc.tile_pool(name="ps", bufs=6, space="PSUM") as ps_pool,
            tc.tile_pool(name="sim", bufs=6) as sim_pool,
            tc.tile_pool(name="acc", bufs=1) as acc_pool,
        ):
            # resident x (stationary operand), k-tiles side by side
            if f16:
                xhi_sb = xpool.tile([128, KT * B], F16, tag="xhi")
                xlo_sb = xpool.tile([128, KT * B], F16, tag="xlo")
                for k in range(KT):
                    nc.sync.dma_start(xhi_sb[:, k * B:(k + 1) * B],
                                      d_xhi[k * 128:(k + 1) * 128, :])
                    nc.sync.dma_start(xlo_sb[:, k * B:(k + 1) * B],
                                      d_xlo[k * 128:(k + 1) * 128, :])
            else:
                xt_sb = xpool.tile([128, KT * B], in_dt, tag="xt")
                for k in range(KT):
                    nc.sync.dma_start(xt_sb[:, k * B:(k + 1) * B],
                                      d_xt[k * 128:(k + 1) * 128, :])

            # result accumulators, [128, QT*NOUT], column q*NOUT + c*8 + j
            vals_sb = acc_pool.tile([128, QT * NOUT], F32, tag="vacc")
            idx_sb = acc_pool.tile([128, QT * NOUT], U32, tag="iacc")

            for c in range(NCHUNK):
                c0 = c * CHUNK
                if f16:
                    ehi_sb = epool.tile([128, KT * CHUNK], F16, tag="ehi")
                    elo_sb = epool.tile([128, KT * CHUNK], F16, tag="elo")
                    for k in range(KT):
                        nc.sync.dma_start(ehi_sb[:, k * CHUNK:(k + 1) * CHUNK],
                                          d_ehi[k * 128:(k + 1) * 128, c0:c0 + CHUNK])
                        nc.sync.dma_start(elo_sb[:, k * CHUNK:(k + 1) * CHUNK],
                                          d_elo[k * 128:(k + 1) * 128, c0:c0 + CHUNK])
                else:
                    en_sb = epool.tile([128, KT * CHUNK], in_dt, tag="en")
                    for k in range(KT):
                        nc.sync.dma_start(en_sb[:, k * CHUNK:(k + 1) * CHUNK],
                                          d_ent[k * 128:(k + 1) * 128, c0:c0 + CHUNK])

                for q in range(QT):
                    ps = ps_pool.tile([128, CHUNK], F32, tag="ps")
                    if variant == "f16x3":
                        nmm = 3 * KT
                        i = 0
                        for k in range(KT):
                            xh = xhi_sb[:, k * B + q * 128: k * B + (q + 1) * 128]
                            xl = xlo_sb[:, k * B + q * 128: k * B + (q + 1) * 128]
                            eh = ehi_sb[:, k * CHUNK:(k + 1) * CHUNK]
                            el = elo_sb[:, k * CHUNK:(k + 1) * CHUNK]
                            for (a, bb) in ((xh, eh), (xh, el), (xl, eh)):
                                nc.tensor.matmul(ps[:, :], a, bb,
                                                 start=(i == 0), stop=(i == nmm - 1))
                                i += 1
                    else:
                        for k in range(KT):
                            lhsT = xt_sb[:, k * B + q * 128: k * B + (q + 1) * 128]
                            rhs = en_sb[:, k * CHUNK:(k + 1) * CHUNK]
                            nc.tensor.matmul(ps[:, :], lhsT, rhs,
                                             start=(k == 0), stop=(k == KT - 1))

                    sim = sim_pool.tile([128, CHUNK], F32, tag="sim")
                    nc.scalar.activation(sim[:, :], ps[:, :], Copy)

                    o = q * NOUT + c * 8
                    nc.vector.max(vals_sb[:, o:o + 8], sim[:, :])
                    nc.vector.max_index(idx_sb[:, o:o + 8], vals_sb[:, o:o + 8],
                                        sim[:, :])

            for q in range(QT):
                nc.sync.dma_start(d_vals[q * 128:(q + 1) * 128, :],
                                  vals_sb[:, q * NOUT:(q + 1) * NOUT])
                nc.sync.dma_start(d_idx[q * 128:(q + 1) * 128, :],
                                  idx_sb[:, q * NOUT:(q + 1) * NOUT])

    nc.compile()
    return nc


def _build_f16w(nc):
    """fp16 single-pass matmul; per-tile 16-wide window max (DVE reduce,
    PSUM-direct); per-core-half top-16 windows per query via
    max/match_replace (first half's selection overlaps the main loop);
    host rescores the selected windows exactly."""
    Max = mybir.AluOpType.max
    X = mybir.AxisListType.X

    d_xh = nc.dram_tensor("xh", [D, B], F16, kind="ExternalInput")
    d_eh = nc.dram_tensor("eh", [D, N_CORE], F16, kind="ExternalInput")
    d_wvals = nc.dram_tensor("wvals", [B, 2 * NSEL], F32, kind="ExternalOutput")
    d_widx = nc.dram_tensor("widx", [B, 2 * NSEL], U32, kind="ExternalOutput")

    # chunk layout: 12 x 1024 + 1 x 512 = 12800
    chunks = [(i * BIGCHUNK, BIGCHUNK) for i in range(N_CORE // BIGCHUNK)]
    rem = N_CORE - (N_CORE // BIGCHUNK) * BIGCHUNK
    if rem:
        chunks.append((N_CORE - rem, rem))
    # selection halves aligned to chunk boundaries:
    # half A = chunks 0-6 (448 windows), half B = chunks 7-12 (352 windows)
    HALF_B = WPC - HALF_A

    def select(wq, vout, iout, o, width, mr_pool):
        nc.vector.max(vout[:, o:o + 8], wq)
        nc.vector.max_index(iout[:, o:o + 8], vout[:, o:o + 8], wq)
        mr = mr_pool.tile([128, width], F32, tag="mr")
        nc.vector.match_replace(mr[:, :width], vout[:, o:o + 8], wq, -1e30)
        nc.vector.max(vout[:, o + 8:o + 16], mr[:, :width])
        nc.vector.max_index(iout[:, o + 8:o + 16],
                            vout[:, o + 8:o + 16], mr[:, :width])

    with tile.TileContext(nc) as tc:
        with (
            tc.tile_pool(name="xpool", bufs=1) as xpool,
            tc.tile_pool(name="epool", bufs=3) as epool,
            tc.tile_pool(name="ps", bufs=3, space="PSUM") as ps_pool,
            tc.tile_pool(name="wacc", bufs=1) as wacc_pool,
            tc.tile_pool(name="mrp", bufs=4) as mr_pool,
            tc.tile_pool(name="outp", bufs=1) as out_pool,
        ):
            xh_sb = xpool.tile([128, KT * B], F16, tag="xh")
            for k in range(KT):
                nc.sync.dma_start(xh_sb[:, k * B:(k + 1) * B],
                                  d_xh[k * 128:(k + 1) * 128, :])

            wmax_sb = wacc_pool.tile([128, QT * WPC], F32, tag="wacc")
            vout_sb = out_pool.tile([128, QT * 2 * NSEL], F32, tag="vout")
            iout_sb = out_pool.tile([128, QT * 2 * NSEL], U32, tag="iout")

            for ci, (c0, cw) in enumerate(chunks):
                eh_sb = epool.tile([128, KT * BIGCHUNK], F16, tag="eh")
                for k in range(KT):
                    nc.sync.dma_start(eh_sb[:, k * cw:(k + 1) * cw],
                                      d_eh[k * 128:(k + 1) * 128, c0:c0 + cw])
                for q in range(QT):
                    ps = ps_pool.tile([128, BIGCHUNK], F32, tag="ps")
                    for s in range(cw // 512):
                        for k in range(KT):
                            nc.tensor.matmul(
                                ps[:, s * 512:(s + 1) * 512],
                                xh_sb[:, k * B + q * 128: k * B + (q + 1) * 128],
                                eh_sb[:, k * cw + s * 512: k * cw + s * 512 + 512],
                                start=(k == 0), stop=(k == KT - 1))
                    nwin = cw // WWIN
                    wslot = q * WPC + c0 // WWIN
                    nc.vector.tensor_reduce(
                        wmax_sb[:, wslot:wslot + nwin],
                        ps[:, :cw].rearrange("p (w i) -> p w i", i=WWIN),
                        axis=X, op=Max)
                # half A (windows [0, HALF_A)) is complete after chunk 6;
                # spread its per-q selection over chunks 6..12 (2-3 q each)
                if ci >= 6:
                    n_grp = len(chunks) - 6
                    qs = [q for q in range(QT) if q % n_grp == ci - 6]
                    for q in qs:
                        select(wmax_sb[:, q * WPC:q * WPC + HALF_A],
                               vout_sb, iout_sb, q * 2 * NSEL, HALF_A, mr_pool)

            for q in range(QT):  # half B (windows [HALF_A, WPC))
                select(wmax_sb[:, q * WPC + HALF_A:(q + 1) * WPC],
                       vout_sb, iout_sb, q * 2 * NSEL + NSEL, HALF_B, mr_pool)

            for q in range(QT):
                nc.sync.dma_start(d_wvals[q * 128:(q + 1) * 128, :],
                                  vout_sb[:, q * 2 * NSEL:(q + 1) * 2 * NSEL])
                nc.sync.dma_start(d_widx[q * 128:(q + 1) * 128, :],
                                  iout_sb[:, q * 2 * NSEL:(q + 1) * 2 * NSEL])

    nc.compile()
    return nc


def _build_f8w(nc):
    """Same structure as f16w, but fp8e4m3 DoubleRow matmuls: operands carry
    [partition, j(2), cols] APs; each matmul contracts 256 dims (2 k-groups
    of 128), so K=512 takes 2 matmuls per 512-wide output slice."""
    Max = mybir.AluOpType.max
    X = mybir.AxisListType.X
    F8 = mybir.dt.float8e4
    DR = mybir.MatmulPerfMode.DoubleRow

    d_x8 = nc.dram_tensor("x8", [D, B], F8, kind="ExternalInput")
    d_e8 = nc.dram_tensor("e8", [D, N_CORE], F8, kind="ExternalInput")
    d_wvals = nc.dram_tensor("wvals", [B, 2 * NSEL], F32, kind="ExternalOutput")
    d_widx = nc.dram_tensor("widx", [B, 2 * NSEL], U32, kind="ExternalOutput")

    chunks = [(i * BIGCHUNK, BIGCHUNK) for i in range(N_CORE // BIGCHUNK)]
    rem = N_CORE - (N_CORE // BIGCHUNK) * BIGCHUNK
    if rem:
        chunks.append((N_CORE - rem, rem))
    HALF_B = WPC - HALF_A

    def select(wq, vout, iout, o, width, mr_pool):
        nc.vector.max(vout[:, o:o + 8], wq)
        nc.vector.max_index(iout[:, o:o + 8], vout[:, o:o + 8], wq)
        mr = mr_pool.tile([128, width], F32, tag="mr")
        nc.vector.match_replace(mr[:, :width], vout[:, o:o + 8], wq, -1e30)
        nc.vector.max(vout[:, o + 8:o + 16], mr[:, :width])
        nc.vector.max_index(iout[:, o + 8:o + 16],
                            vout[:, o + 8:o + 16], mr[:, :width])

    with tile.TileContext(nc) as tc:
        with (
            tc.tile_pool(name="xpool", bufs=1) as xpool,
            tc.tile_pool(name="epool", bufs=3) as epool,
            tc.tile_pool(name="ps", bufs=3, space="PSUM") as ps_pool,
            tc.tile_pool(name="wacc", bufs=1) as wacc_pool,
            tc.tile_pool(name="mrp", bufs=4) as mr_pool,
            tc.tile_pool(name="outp", bufs=1) as out_pool,
        ):
            # [g][j][cols] layout: row-range g*256 + j*128 of the [D, *] input
            x_sb = xpool.tile([128, 4 * B], F8, tag="x8")
            for g in range(2):
                for j in range(2):
                    r0 = g * 256 + j * 128
                    nc.sync.dma_start(x_sb[:, (g * 2 + j) * B:(g * 2 + j + 1) * B],
                                      d_x8[r0:r0 + 128, :])

            wmax_sb = wacc_pool.tile([128, QT * WPC], F32, tag="wacc")
            vout_sb = out_pool.tile([128, QT * 2 * NSEL], F32, tag="vout")
            iout_sb = out_pool.tile([128, QT * 2 * NSEL], U32, tag="iout")

            for ci, (c0, cw) in enumerate(chunks):
                eh_sb = epool.tile([128, 4 * BIGCHUNK], F8, tag="e8")
                for g in range(2):
                    for j in range(2):
                        r0 = g * 256 + j * 128
                        nc.sync.dma_start(
                            eh_sb[:, (g * 2 + j) * cw:(g * 2 + j + 1) * cw],
                            d_e8[r0:r0 + 128, c0:c0 + cw])
                for q in range(QT):
                    ps = ps_pool.tile([128, BIGCHUNK], F32, tag="ps")
                    for s in range(cw // 512):
                        for g in range(2):
                            lhsT = x_sb[:, g * 2 * B:(g + 1) * 2 * B].rearrange(
                                "p (j b) -> p j b", j=2)[:, :, q * 128:(q + 1) * 128]
                            rhs = eh_sb[:, g * 2 * cw:(g + 1) * 2 * cw].rearrange(
                                "p (j n) -> p j n", j=2)[:, :, s * 512:(s + 1) * 512]
                            nc.tensor.matmul(ps[:, s * 512:(s + 1) * 512],
                                             lhsT, rhs, perf_mode=DR,
                                             start=(g == 0), stop=(g == 1))
                    nwin = cw // WWIN
                    wslot = q * WPC + c0 // WWIN
                    nc.vector.tensor_reduce(
                        wmax_sb[:, wslot:wslot + nwin],
                        ps[:, :cw].rearrange("p (w i) -> p w i", i=WWIN),
                        axis=X, op=Max)
                if ci >= 6:
                    n_grp = len(chunks) - 6
                    qs = [q for q in range(QT) if q % n_grp == ci - 6]
                    for q in qs:
                        select(wmax_sb[:, q * WPC:q * WPC + HALF_A],
                               vout_sb, iout_sb, q * 2 * NSEL, HALF_A, mr_pool)

            for q in range(QT):
                select(wmax_sb[:, q * WPC + HALF_A:(q + 1) * WPC],
                       vout_sb, iout_sb, q * 2 * NSEL + NSEL, HALF_B, mr_pool)

            for q in range(QT):
                nc.sync.dma_start(d_wvals[q * 128:(q + 1) * 128, :],
                                  vout_sb[:, q * 2 * NSEL:(q + 1) * 2 * NSEL])
                nc.sync.dma_start(d_widx[q * 128:(q + 1) * 128, :],
                                  iout_sb[:, q * 2 * NSEL:(q + 1) * 2 * NSEL])

    nc.compile()
    return nc


_F8_LUT = None


def _to_f8(a):
    """Fast float->fp8e4m3: fp16 hardware cast, then a 64K-entry LUT over the
    fp16 bit patterns (ml_dtypes' elementwise astype is ~50x slower). The
    double rounding vs a direct fp32->fp8 cast is harmless here: any
    consistent rounding is covered by the selection margin."""
    global _F8_LUT
    import ml_dtypes
    if _F8_LUT is None:
        with np.errstate(all="ignore"):
            all16 = np.arange(65536, dtype=np.uint16).view(np.float16)
            _F8_LUT = (all16.astype(np.float32)
                       .astype(ml_dtypes.float8_e4m3).view(np.uint8))
    h = a.astype(np.float16).view(np.uint16)
    return _F8_LUT[h].view(ml_dtypes.float8_e4m3)


def _build_f8d(nc):
    """f8w minus on-device window selection: the full per-window max array
    ships to the host (3.3MB/core), which does the margin selection itself.
    ScalarE stages PSUM->SBUF so the DVE reduce pays the SBUF (not PSUM)
    access bubble; DVE runs nothing but the 208 window-max reduces."""
    Max = mybir.AluOpType.max
    X = mybir.AxisListType.X
    F8 = mybir.dt.float8e4
    DR = mybir.MatmulPerfMode.DoubleRow
    Copy = mybir.ActivationFunctionType.Copy

    d_x8 = nc.dram_tensor("x8", [D, B], F8, kind="ExternalInput")
    d_e8 = nc.dram_tensor("e8", [D, N_CORE], F8, kind="ExternalInput")
    d_wmax = nc.dram_tensor("wmax", [B, WPC], F32, kind="ExternalOutput")

    chunks = [(i * BIGCHUNK, BIGCHUNK) for i in range(N_CORE // BIGCHUNK)]
    rem = N_CORE - (N_CORE // BIGCHUNK) * BIGCHUNK
    if rem:
        chunks.append((N_CORE - rem, rem))

    with tile.TileContext(nc) as tc:
        with (
            tc.tile_pool(name="xpool", bufs=1) as xpool,
            tc.tile_pool(name="epool", bufs=3) as epool,
            tc.tile_pool(name="ps", bufs=3, space="PSUM") as ps_pool,
            tc.tile_pool(name="stg", bufs=3) as stg_pool,
            tc.tile_pool(name="wacc", bufs=1) as wacc_pool,
        ):
            x_sb = xpool.tile([128, 4 * B], F8, tag="x8")
            for g in range(2):
                for j in range(2):
                    r0 = g * 256 + j * 128
                    nc.sync.dma_start(x_sb[:, (g * 2 + j) * B:(g * 2 + j + 1) * B],
                                      d_x8[r0:r0 + 128, :])

            wmax_sb = wacc_pool.tile([128, QT * WPC], F32, tag="wacc")

            for (c0, cw) in chunks:
                eh_sb = epool.tile([128, 4 * BIGCHUNK], F8, tag="e8")
                for g in range(2):
                    for j in range(2):
                        r0 = g * 256 + j * 128
                        nc.sync.dma_start(
                            eh_sb[:, (g * 2 + j) * cw:(g * 2 + j + 1) * cw],
                            d_e8[r0:r0 + 128, c0:c0 + cw])
                for q in range(QT):
                    ps = ps_pool.tile([128, BIGCHUNK], F32, tag="ps")
                    for s in range(cw // 512):
                        for g in range(2):
                            lhsT = x_sb[:, g * 2 * B:(g + 1) * 2 * B].rearrange(
                                "p (j b) -> p j b", j=2)[:, :, q * 128:(q + 1) * 128]
                            rhs = eh_sb[:, g * 2 * cw:(g + 1) * 2 * cw].rearrange(
                                "p (j n) -> p j n", j=2)[:, :, s * 512:(s + 1) * 512]
                            nc.tensor.matmul(ps[:, s * 512:(s + 1) * 512],
                                             lhsT, rhs, perf_mode=DR,
                                             start=(g == 0), stop=(g == 1))
                    stg = stg_pool.tile([128, BIGCHUNK], F32, tag="stg")
                    nc.scalar.activation(stg[:, :cw], ps[:, :cw], Copy)
                    nwin = cw // WWIN
                    wslot = q * WPC + c0 // WWIN
                    nc.vector.tensor_reduce(
                        wmax_sb[:, wslot:wslot + nwin],
                        stg[:, :cw].rearrange("p (w i) -> p w i", i=WWIN),
                        axis=X, op=Max)

            for q in range(QT):
                nc.sync.dma_start(d_wmax[q * 128:(q + 1) * 128, :],
                                  wmax_sb[:, q * WPC:(q + 1) * WPC])

    nc.compile()
    return nc


def _build_f8e(nc):
    """f8d with wider DVE reduces (two staged PSUM tiles -> one 2048-wide
    window-max, halving the per-op SBUF bubble count) and per-half early
    wmax DMA-out so the output transfer overlaps the main loop."""
    Max = mybir.AluOpType.max
    X = mybir.AxisListType.X
    F8 = mybir.dt.float8e4
    DR = mybir.MatmulPerfMode.DoubleRow
    Copy = mybir.ActivationFunctionType.Copy

    d_x8 = nc.dram_tensor("x8", [D, B], F8, kind="ExternalInput")
    d_e8 = nc.dram_tensor("e8", [D, N_CORE], F8, kind="ExternalInput")
    d_wmax = nc.dram_tensor("wmax", [B, WPC], F32, kind="ExternalOutput")

    BC = 2048  # 4 PSUM banks per tile; 6x2048 + 1x512 = 12800
    chunks = [(i * BC, BC) for i in range(N_CORE // BC)]
    rem = N_CORE - (N_CORE // BC) * BC
    if rem:
        chunks.append((N_CORE - rem, rem))
    AWIN = (4 * BC) // WWIN  # 256 windows (chunks 0-3) ship mid-loop

    with tile.TileContext(nc) as tc:
        with (
            tc.tile_pool(name="xpool", bufs=1) as xpool,
            tc.tile_pool(name="epool", bufs=3) as epool,
            tc.tile_pool(name="ps", bufs=2, space="PSUM") as ps_pool,
            tc.tile_pool(name="stg", bufs=3) as stg_pool,
            tc.tile_pool(name="wacc", bufs=1) as wacc_pool,
        ):
            x_sb = xpool.tile([128, 4 * B], F8, tag="x8")
            for g in range(2):
                for j in range(2):
                    r0 = g * 256 + j * 128
                    nc.sync.dma_start(x_sb[:, (g * 2 + j) * B:(g * 2 + j + 1) * B],
                                      d_x8[r0:r0 + 128, :])

            wmax_sb = wacc_pool.tile([128, QT * WPC], F32, tag="wacc")

            for ci, (c0, cw) in enumerate(chunks):
                eh_sb = epool.tile([128, 4 * BC], F8, tag="e8")
                for g in range(2):
                    for j in range(2):
                        r0 = g * 256 + j * 128
                        nc.sync.dma_start(
                            eh_sb[:, (g * 2 + j) * cw:(g * 2 + j + 1) * cw],
                            d_e8[r0:r0 + 128, c0:c0 + cw])
                for q in range(QT):
                    ps = ps_pool.tile([128, BC], F32, tag="ps")
                    for s0 in range(0, cw, 512):
                        sw = min(512, cw - s0)
                        for g in range(2):
                            lhsT = x_sb[:, g * 2 * B:(g + 1) * 2 * B].rearrange(
                                "p (j b) -> p j b", j=2)[:, :, q * 128:(q + 1) * 128]
                            rhs = eh_sb[:, g * 2 * cw:(g + 1) * 2 * cw].rearrange(
                                "p (j n) -> p j n", j=2)[:, :, s0:s0 + sw]
                            nc.tensor.matmul(ps[:, s0:s0 + sw],
                                             lhsT, rhs, perf_mode=DR,
                                             start=(g == 0), stop=(g == 1))
                    stg = stg_pool.tile([128, BC], F32, tag="stg")
                    nc.scalar.activation(stg[:, :cw], ps[:, :cw], Copy)
                    nwin = cw // WWIN
                    wslot = q * WPC + c0 // WWIN
                    nc.vector.tensor_reduce(
                        wmax_sb[:, wslot:wslot + nwin],
                        stg[:, :cw].rearrange("p (w i) -> p w i", i=WWIN),
                        axis=X, op=Max)
                    if ci == 3:  # chunks 0-3 reduced for q: ship 256 windows
                        nc.sync.dma_start(
                            d_wmax[q * 128:(q + 1) * 128, :AWIN],
                            wmax_sb[:, q * WPC:q * WPC + AWIN])

            for q in range(QT):
                nc.sync.dma_start(d_wmax[q * 128:(q + 1) * 128, AWIN:],
                                  wmax_sb[:, q * WPC + AWIN:(q + 1) * WPC])

    nc.compile()
    return nc


def _prep_f8w(xn, e, inv):
    """in_maps for the f8w variant: fp8e4m3 transposed normalized shards,
    scaled by F8_SCALE to stay clear of the fp8 subnormal range."""
    import ml_dtypes
    f8 = ml_dtypes.float8_e4m3
    x8 = _to_f8(np.ascontiguousarray(xn.T) * np.float32(F8_SCALE))
    in_maps = []
    for i in range(CORES):
        lo_r, hi_r = i * N_CORE, (i + 1) * N_CORE
        n_real = max(0, min(hi_r, N_EMB) - lo_r)
        e8 = np.zeros((D, N_CORE), dtype=f8)
        if n_real > 0:
            sl = e[lo_r:lo_r + n_real] * (inv[lo_r:lo_r + n_real]
                                          * np.float32(F8_SCALE))[:, None]
            e8[:, :n_real] = _to_f8(sl.T)
        in_maps.append({"x8": x8, "e8": e8})
    return in_maps


def _get_nc(variant=None):
    variant = variant or MM_DTYPE
    if variant not in _CACHE:
        _CACHE[variant] = _build(variant)
    return _CACHE[variant]


def _normalize(x, embeddings):
    x = np.asarray(x, dtype=np.float32)
    e = np.asarray(embeddings, dtype=np.float32)
    xn = x / np.maximum(np.linalg.norm(x, axis=1, keepdims=True), EPS)
    inv = (1.0 / np.maximum(np.linalg.norm(e, axis=1), EPS)).astype(np.float32)
    return xn, e, inv


def _prep_f16w(xn, e, inv):
    """in_maps for the f16w variant: fp16 transposed normalized shards."""
    xh = np.ascontiguousarray(xn.T).astype(np.float16)
    in_maps = []
    for i in range(CORES):
        lo_r, hi_r = i * N_CORE, (i + 1) * N_CORE
        n_real = max(0, min(hi_r, N_EMB) - lo_r)
        eh = np.zeros((D, N_CORE), dtype=np.float16)
        if n_real > 0:
            sl = e[lo_r:lo_r + n_real] * inv[lo_r:lo_r + n_real][:, None]
            eh[:, :n_real] = sl.T.astype(np.float16)
        in_maps.append({"xh": xh, "eh": eh})
    return in_maps


def _prep_inputs(x, embeddings, variant):
    """Host prep: normalize embeddings, pad, transpose, shard; returns in_maps.

    Works per-core-shard to keep intermediates cache-sized."""
    if variant == "f16w":
        xn, e, inv = _normalize(x, embeddings)
        return _prep_f16w(xn, e, inv)
    if variant in ("f8w", "f8d", "f8e"):
        xn, e, inv = _normalize(x, embeddings)
        return _prep_f8w(xn, e, inv)
    x = np.asarray(x, dtype=np.float32)
    e = np.asarray(embeddings, dtype=np.float32)
    inv = (1.0 / np.maximum(np.linalg.norm(e, axis=1), EPS)).astype(np.float32)
    xt = np.ascontiguousarray(x.T)               # [D, B]

    in_maps = []
    for i in range(CORES):
        lo_r, hi_r = i * N_CORE, (i + 1) * N_CORE
        n_real = max(0, min(hi_r, N_EMB) - lo_r)
        ent = np.zeros((D, N_CORE), dtype=np.float32)
        if n_real > 0:
            sl = e[lo_r:lo_r + n_real]
            ent[:, :n_real] = sl.T * inv[lo_r:lo_r + n_real][None, :]
        if variant == "f16x3":
            ehi = ent.astype(np.float16)
            elo = (ent - ehi).astype(np.float16)
            in_maps.append({"ehi": ehi, "elo": elo})
        else:
            in_maps.append({"ent": ent})

    if variant == "f16x3":
        xhi = xt.astype(np.float16)
        xlo = (xt - xhi).astype(np.float16)
        for m in in_maps:
            m["xhi"] = xhi
            m["xlo"] = xlo
    else:
        for m in in_maps:
            m["xt"] = xt
    return in_maps


def _merge(results, labels):
    """Host merge: exact global top-10 from per-core per-chunk top-8 pools,
    then the reference's mode computation."""
    vals = np.concatenate([r["vals"] for r in results], axis=1)   # [B, 8*NOUT]
    idx8 = np.concatenate([r["idx"] for r in results], axis=1).astype(np.int64)

    col_base = (np.arange(NOUT, dtype=np.int64) // 8) * CHUNK      # chunk offset
    core_base = np.repeat(np.arange(CORES, dtype=np.int64) * N_CORE, NOUT)
    g = idx8 + np.tile(col_base, CORES)[None, :] + core_base[None, :]

    # padding rows (g >= N_EMB) are zero embeddings: exclude
    u = vals.view(np.uint32)
    key = np.where(u & 0x80000000, ~u, u | 0x80000000).astype(np.uint64)
    combo = ((np.uint64(0xFFFFFFFF) - key) << np.uint64(17)) | g.astype(np.uint64)
    combo[g >= N_EMB] = np.uint64(0xFFFFFFFFFFFFFFFF)
    order = np.argsort(combo, axis=1, kind="stable")[:, :K_NEIGH]
    neighbors = np.take_along_axis(g, order, axis=1)               # [B, 10]

    labels = np.asarray(labels)
    nl = labels[neighbors].astype(np.int64)                        # [B, 10]
    eq = nl[:, :, None] == nl[:, None, :]
    counts = eq.sum(-1)
    mkey = counts * (NUM_CLASSES + 1) + (NUM_CLASSES - nl)
    mi = np.argmax(mkey, axis=1)
    pred = np.take_along_axis(nl, mi[:, None], axis=1)[:, 0]
    return pred.astype(labels.dtype)


class _Runner:
    """Caches the shard_map-jitted executable across calls (mirrors
    bass2jax.run_bass_via_pjrt's multi-core path, which re-traces per call)."""

    def __init__(self, variant):
        import jax
        import concourse.mybir as mb
        from concourse import bass2jax
        from jax.experimental.shard_map import shard_map
        from jax.sharding import Mesh, PartitionSpec

        bass2jax.install_neuronx_cc_hook()
        self.jax = jax
        nc = _get_nc(variant)
        partition_name = (nc.partition_id_tensor.name
                          if nc.partition_id_tensor else None)
        in_names, out_names, out_avals, zeros = [], [], [], []
        for alloc in nc.m.functions[0].allocations:
            if not isinstance(alloc, mb.MemoryLocationSet):
                continue
            name = alloc.memorylocations[0].name
            if alloc.kind == "ExternalInput":
                if name != partition_name:
                    in_names.append(name)
            elif alloc.kind == "ExternalOutput":
                shape = tuple(alloc.tensor_shape)
                dtype = mb.dt.np(alloc.dtype)
                out_avals.append(jax.core.ShapedArray(shape, dtype))
                out_names.append(name)
                zeros.append(np.zeros((CORES * shape[0],) + shape[1:], dtype))
        self.in_names = list(in_names)
        self.out_names = out_names
        self.out_avals = out_avals
        self.zeros = zeros
        n_params = len(in_names)
        all_names = in_names + out_names
        if partition_name is not None:
            all_names = all_names + [partition_name]
        donate = tuple(range(n_params, n_params + len(out_names)))

        def _body(*args):
            operands = list(args)
            if partition_name is not None:
                operands.append(bass2jax.partition_id_tensor())
            outs = bass2jax._bass_exec_p.bind(
                *operands,
                out_avals=tuple(out_avals),
                in_names=tuple(all_names),
                out_names=tuple(out_names),
                lowering_input_output_aliases=(),
                sim_require_finite=True,
                sim_require_nnan=True,
                nc=nc,
            )
            return tuple(outs)

        devices = jax.devices()[:CORES]
        self.mesh = Mesh(np.asarray(devices), ("core",))
        self.pspec = PartitionSpec("core")
        in_specs = (self.pspec,) * (n_params + len(out_names))
        out_specs = (self.pspec,) * len(out_names)
        self.sharded = jax.jit(
            shard_map(_body, mesh=self.mesh, in_specs=in_specs,
                      out_specs=out_specs, check_rep=False),
            donate_argnums=donate, keep_unused=True,
        )

    def concat_inputs(self, in_maps):
        return [
            np.concatenate([np.asarray(m[name]) for m in in_maps], axis=0)
            for name in self.in_names
        ]

    def device_put(self, concat_in):
        from jax.sharding import NamedSharding
        sh = NamedSharding(self.mesh, self.pspec)
        return [self.jax.device_put(a, sh) for a in concat_in]

    def execute(self, concat_in):
        zeros = [np.zeros_like(z) for z in self.zeros]
        out_arrs = self.sharded(*concat_in, *zeros)
        return out_arrs

    def run(self, in_maps):
        out_arrs = self.execute(self.concat_inputs(in_maps))
        return [
            {
                name: np.asarray(out_arrs[i]).reshape(
                    CORES, *self.out_avals[i].shape)[c]
                for i, name in enumerate(self.out_names)
            }
            for c in range(CORES)
        ]


_RUNNERS = {}


def _get_runner(variant=None):
    variant = variant or MM_DTYPE
    if variant not in _RUNNERS:
        _RUNNERS[variant] = _Runner(variant)
    return _RUNNERS[variant]


def _mode_pred(neighbors, labels):
    """Reference's torch.mode semantics on gathered neighbor labels."""
    labels = np.asarray(labels)
    nl = labels[neighbors].astype(np.int64)                        # [B, 10]
    eq = nl[:, :, None] == nl[:, None, :]
    counts = eq.sum(-1)
    mkey = counts * (NUM_CLASSES + 1) + (NUM_CLASSES - nl)
    mi = np.argmax(mkey, axis=1)
    pred = np.take_along_axis(nl, mi[:, None], axis=1)[:, 0]
    return pred.astype(labels.dtype)


def _merge_f16w(results, labels, xn, e, inv, margin=MARGIN):
    """Select windows >= (10th-best window max) - margin, rescore those
    candidates exactly in fp64, exact global top-10, then mode."""
    wv = np.stack([r["wvals"] for r in results], axis=1)      # [B, 8, 32]
    wi = np.stack([r["widx"] for r in results], axis=1).astype(np.int64)
    wi[:, :, NSEL:] += HALF_A   # half-B indices are relative to its slice
    gw = wi + (np.arange(CORES, dtype=np.int64) * WPC)[None, :, None]
    wv = wv.reshape(B, CORES * 2 * NSEL)
    gw = gw.reshape(B, CORES * 2 * NSEL)

    w10 = np.partition(wv, wv.shape[1] - K_NEIGH, axis=1)[:, wv.shape[1] - K_NEIGH]
    keep = wv >= (w10[:, None] - margin)
    smax = int(keep.sum(axis=1).max())

    # top-smax windows per row by value; mask out ones below the cutoff
    order = np.argsort(-wv, axis=1, kind="stable")[:, :smax]
    sel_g = np.take_along_axis(gw, order, axis=1)              # [B, smax]
    sel_keep = np.take_along_axis(keep, order, axis=1)

    # rescore grouped by window: each window's embeddings are one contiguous
    # 32-row slice, shared by every query that selected it (~6400 windows
    # total vs ~170k (row, window) pairs -> tiny gathers, BLAS-sized GEMMs)
    e = np.asarray(e, dtype=np.float32)
    xn32 = np.ascontiguousarray(xn, dtype=np.float32)
    rows_idx, slots = np.nonzero(sel_keep)
    wins = sel_g[rows_idx, slots]
    order = np.argsort(wins, kind="stable")
    rows_idx, slots, wins = rows_idx[order], slots[order], wins[order]
    uniq, starts = np.unique(wins, return_index=True)
    bounds = np.append(starts, len(wins))

    sims = np.full((B, smax, WWIN), -np.inf, dtype=np.float32)
    for ui in range(len(uniq)):
        w = int(uniq[ui])
        c0, c1 = w * WWIN, min(w * WWIN + WWIN, N_EMB)
        if c1 <= c0:
            continue
        s0, s1 = bounds[ui], bounds[ui + 1]
        en_w = e[c0:c1] * inv[c0:c1][:, None]                  # [<=32, D]
        sblk = xn32[rows_idx[s0:s1]] @ en_w.T                  # [nrows, <=32]
        sims[rows_idx[s0:s1], slots[s0:s1], :c1 - c0] = sblk

    cand = (sel_g[:, :, None] * WWIN +
            np.arange(WWIN, dtype=np.int64)[None, None, :]).reshape(B, -1)
    sims = sims.reshape(B, -1)

    # exact top-10 by (-sim, cand) via an order-preserving uint64 key
    u = sims.view(np.uint32)
    mono = np.where(u & 0x80000000, ~u, u | 0x80000000).astype(np.uint64)
    combo = ((np.uint64(0xFFFFFFFF) - mono) << np.uint64(17)) | \
        cand.astype(np.uint64)
    combo[sims == -np.inf] = np.uint64(0xFFFFFFFFFFFFFFFF)
    ordr = np.argsort(combo, axis=1, kind="stable")[:, :K_NEIGH]
    neighbors = np.take_along_axis(cand, ordr, axis=1)
    return _mode_pred(neighbors, labels)


def _merge_f8d(results, labels, xn, e, inv, margin):
    """Host-side window selection from the full per-window-max arrays, then
    the window-grouped exact rescore."""
    wv = np.concatenate([r["wmax"] for r in results], axis=1)   # [B, 8*WPC]
    nw = wv.shape[1]
    w10 = np.partition(wv, nw - K_NEIGH, axis=1)[:, nw - K_NEIGH]
    keep = wv >= (w10[:, None] - margin)                        # [B, 8*WPC]

    rows_idx, wins = np.nonzero(keep)        # wins are global window ids
    slots = (np.cumsum(keep, axis=1) - 1)[rows_idx, wins]
    smax = int(keep.sum(axis=1).max())

    e = np.asarray(e, dtype=np.float32)
    xn32 = np.ascontiguousarray(xn, dtype=np.float32)
    order = np.argsort(wins, kind="stable")
    rows_s, slots_s, wins_s = rows_idx[order], slots[order], wins[order]
    uniq, starts = np.unique(wins_s, return_index=True)
    bounds = np.append(starts, len(wins_s))

    sims = np.full((B, smax, WWIN), -np.inf, dtype=np.float32)
    wfull = np.zeros((B, smax), dtype=np.int64)
    wfull[rows_idx, slots] = wins
    for ui in range(len(uniq)):
        w = int(uniq[ui])
        c0, c1 = w * WWIN, min(w * WWIN + WWIN, N_EMB)
        if c1 <= c0:
            continue
        s0, s1 = bounds[ui], bounds[ui + 1]
        en_w = e[c0:c1] * inv[c0:c1][:, None]
        sblk = xn32[rows_s[s0:s1]] @ en_w.T
        sims[rows_s[s0:s1], slots_s[s0:s1], :c1 - c0] = sblk

    cand = (wfull[:, :, None] * WWIN +
            np.arange(WWIN, dtype=np.int64)[None, None, :]).reshape(B, -1)
    sims = sims.reshape(B, -1)
    u = sims.view(np.uint32)
    mono = np.where(u & 0x80000000, ~u, u | 0x80000000).astype(np.uint64)
    combo = ((np.uint64(0xFFFFFFFF) - mono) << np.uint64(17)) | \
        cand.astype(np.uint64)
    combo[sims == -np.inf] = np.uint64(0xFFFFFFFFFFFFFFFF)
    ordr = np.argsort(combo, axis=1, kind="stable")[:, :K_NEIGH]
    neighbors = np.take_along_axis(cand, ordr, axis=1)
    return _mode_pred(neighbors, labels)


def run_on_hw(x, embeddings, variant=None):
    runner = _get_runner(variant)
    in_maps = _prep_inputs(x, embeddings, variant or MM_DTYPE)
    return runner.run(in_maps)


def kernel(x, embeddings, labels):
    variant = MM_DTYPE
    if variant == "f16w":
        xn, e, inv = _normalize(x, embeddings)
        runner = _get_runner(variant)
        results = runner.run(_prep_f16w(xn, e, inv))
        return _merge_f16w(results, labels, xn, e, inv)
    if variant == "f8w":
        xn, e, inv = _normalize(x, embeddings)
        runner = _get_runner(variant)
        results = runner.run(_prep_f8w(xn, e, inv))
        return _merge_f16w(results, labels, xn, e, inv,
                           margin=MARGIN_F8 * F8_SCALE * F8_SCALE)
    if variant in ("f8d", "f8e"):
        xn, e, inv = _normalize(x, embeddings)
        runner = _get_runner(variant)
        results = runner.run(_prep_f8w(xn, e, inv))
        return _merge_f8d(results, labels, xn, e, inv,
                          margin=MARGIN_F8 * F8_SCALE * F8_SCALE)
    results = run_on_hw(x, embeddings)
    return _merge(results, labels)
